# revision 29
# baseline (speedup 1.0000x reference)
"""ARAP loss kernel for Trainium2 (8 NeuronCores, SPMD over the vertex axis).

Problem: nn_ArapLoss — per-vertex 6-neighbor gather on a 316x316 grid mesh,
3x3 polar decomposition (closed-form symmetric eigenanalysis) per vertex,
cotan-weighted edge-residual energy, clamped mean over vertices.

Strategy (v3 — vector-engine lean, pair-pipelined)
--------------------------------------------------
- Shard the vertex axis N=99856 across 8 cores (12482 each, padded to
  12544 = 128*98).  Grid adjacency reduces to K=6 constant index offsets
  {+-1, +-316, +-317}; the host materializes shifted windows of
  `prediction` so the device does no gather.
- Edge vectors e_k = q_{n+o_k} - p_n are computed ONCE in f32 and stored
  bf16; everything downstream runs in bf16 (DVE 2x mode).
- The template-edge xy components are EXACTLY {0,+-1} per offset class
  (regular grid), so A = sum_k e_k (stab w_k t_k)^T collapses to signed
  sums plus one weighted z-column, and the rotated-template residual
  e_k - R t_k collapses to (e_k - tz_k R[:,2]) -+ R-column combos.
- R from a SINGLE 3x3 product:  R = Y + cof(Y),  Y = A (g2 P2 + d g3 P3).
  cof(u2 v2' + d u3 v3') = d^2 u1 v1' = u1 v1', so the smallest-eigenvalue
  component needs no division by s1 and no second product / sign fixup.
- Safe reciprocals as x/(x^2 + eps) — no Abs/Sign, sign rides the x.
- Passes are emitted in PAIRS, interleaved at every ACT-dependency
  boundary, so the in-order DVE queue always has independent work while
  the scalar engine walks its serial ln/exp/arctan/sin chain.  The trig
  and sqrt activations of the two passes share one table-load block.
- Output: per-core partial sums [128, B]; host reduces and divides by N.
"""
import sys

for _p in ("/opt/trn_rl_repo", "/opt/trn_rl_repo/concourse", "/opt/pypackages"):
    if _p not in sys.path:
        sys.path.insert(0, _p)

from types import SimpleNamespace

from contextlib import ExitStack

import ml_dtypes
import numpy as np

import concourse.bass as bass
import concourse.tile as tile
from concourse import bacc, mybir
from concourse.bass_utils import run_bass_kernel_spmd

F32 = mybir.dt.float32
BF = mybir.dt.bfloat16
AL = mybir.AluOpType
AF = mybir.ActivationFunctionType

# ---- problem geometry (hardcoded per spec) --------------------------------
B = 16
NV = 99856
NCORES = 8
P = 128
NC_V = NV // NCORES            # 12482 real vertices per core
FQ = 98                        # free-dim vertices per partition
VP = P * FQ                    # 12544 padded vertices per core
BQ = 4                         # batch elements per pass
NQ = B // BQ
K = 6
CLIPV = 1e-12                  # eigenvalue clamp (unscaled units)
C_SINL = float(2.0 * np.pi / 3.0)
RCLAMP = 1.0 - 1e-6
OFFS = (-317, -316, -1, 1, 316, 317)
# xy components of template edges per offset class (exact on the grid)
CX = (-1, -1, 0, 0, 1, 1)
CY = (-1, 0, -1, 1, 0, 1)

_nc_cache = {}


def _pin_act_tables():
    """Shrink the cached activation-table membership map so the compiler
    assigns Ln/Exp/Square/Sign/Abs/Copy to the one combined set that
    physically contains them all (natural_log_exp_and_others), Sin to
    trig_and_small, Arctan to sigmoid_and_others.  Every set we leave a
    function in really does contain it, so the emitted table loads stay
    valid — this only stops the compiler from ping-ponging between the
    ln-only and exp-only sets.  Best effort: on any surprise, leave the
    tables untouched (costs extra table loads, still correct)."""
    try:
        from concourse.hw_specs import get_activation_tables
        tabs = None
        for arch in ("gen3", "TRN2"):
            try:
                tabs = get_activation_tables(arch)
                break
            except Exception:
                continue
        if tabs is None:
            return
        combined = tabs.get("natural_log_exp_and_others")
        shared = {AF.Ln, AF.Exp, AF.Square, AF.Sign, AF.Abs, AF.Copy,
                  AF.Identity}
        if combined is None or not (shared <= combined):
            return
        if AF.Sin not in tabs.get("trig_and_small", set()):
            return
        for name, fns in tabs.items():
            if name == "natural_log_exp_and_others":
                continue
            fns -= shared
            if name != "trig_and_small":
                fns.discard(AF.Sin)
    except Exception:
        pass


# ---------------------------------------------------------------------------
# Host-side preprocessing
# ---------------------------------------------------------------------------

def _build_offset_classes(adj_idx, adj_w, tev_T):
    """(N,D) adjacency -> per-offset-class weights wk (K,N) and template
    edge z-components tzk (K,N).  Asserts the grid structure this kernel
    hardcodes (xy components == CX/CY per class)."""
    N, D = adj_idx.shape
    ar = np.arange(N, dtype=np.int64)
    real = (adj_idx > 0) | (np.arange(D)[None, :] == 0)
    delta = np.asarray(adj_idx, np.int64) - ar[:, None]
    offs = np.unique(delta[real])
    assert tuple(int(o) for o in offs) == OFFS, f"unexpected offsets {offs}"
    wk = np.zeros((K, N), np.float32)
    tzk = np.zeros((K, N), np.float32)
    for k, o in enumerate(OFFS):
        sel = real & (delta == o)
        n_id, d_id = np.nonzero(sel)
        wk[k, n_id] = adj_w[n_id, d_id]
        tzk[k, n_id] = tev_T[n_id, 2, d_id]
    return wk, tzk


def _group_offsets(gap=8):
    """Group [0]+OFFS into consecutive runs; returns (bases, width, win_map)
    where win_map[x] = (g, slot) for x in [0(center)] + OFFS order."""
    allo = sorted(set([0] + list(OFFS)))
    groups = [[allo[0]]]
    for o in allo[1:]:
        if o - groups[-1][-1] <= gap:
            groups[-1].append(o)
        else:
            groups.append([o])
    bases = [g[0] for g in groups]
    width = FQ + max(g[-1] - g[0] for g in groups) + 1
    lut = {}
    for gi, g in enumerate(groups):
        for o in g:
            lut[o] = (gi, o - g[0])
    win_map = [lut[0]] + [lut[o] for o in OFFS]
    return bases, width, tuple(win_map)


def _host_prepare(pred, wk, tzk):
    """Build per-core input maps: predl [P, G*B*3*GWD] f32 (group-major so
    each pass loads G contiguous chunks) and constb [P, 24*FQ] bf16
    (rows: wp(6), wz(6), tz(6), wk(6))."""
    bases, GWD, win_map = _group_offsets()
    G = len(bases)
    H = max(max(abs(o) for o in OFFS), 1)
    padlen = NV + 2 * H + (VP - NC_V) + GWD
    padG = np.zeros((B, 3, padlen), np.float32)
    padG[:, :, H:H + NV] = pred

    wp = wk                              # (K, N) — no stab scaling: R is
    wz = wp * tzk                        # scale-invariant; bf16 ranges stay sane
    CG = np.concatenate([wp, wz, tzk, wk], axis=0)   # (24, N)

    in_maps = []
    pidx = (np.arange(P)[:, None] * FQ + np.arange(GWD)[None, :])  # (P,GWD)
    for c in range(NCORES):
        base = c * NC_V
        wins = np.empty((G, B, 3, P, GWD), np.float32)
        for g, bg in enumerate(bases):
            idx = H + base + bg + pidx
            wins[g] = padG[:, :, idx].transpose(0, 1, 2, 3)
        predl = np.ascontiguousarray(
            wins.transpose(3, 0, 1, 2, 4)
        ).reshape(P, G * B * 3 * GWD)

        cc = np.zeros((24, VP), np.float32)
        hi = min(base + VP, NV) - base
        hi = min(hi, NC_V)                   # zero weights on padded tail
        cc[:, :hi] = CG[:, base:base + hi]
        constb = np.ascontiguousarray(
            cc.reshape(24, P, FQ).transpose(1, 0, 2)
        ).reshape(P, 24 * FQ).astype(ml_dtypes.bfloat16)

        in_maps.append({"predl": predl, "constb": constb})
    return in_maps, (G, GWD, win_map)


# ---------------------------------------------------------------------------
# Device kernel builder
# ---------------------------------------------------------------------------

def _build_nc(wingeo):
    G, GWD, win_map = wingeo
    FD = BQ * FQ

    nc = bacc.Bacc("TRN2", target_bir_lowering=False, debug=False,
                   num_devices=NCORES)

    predl_d = nc.dram_tensor("predl", [P, G * B * 3 * GWD], F32,
                             kind="ExternalInput").ap()
    constb_d = nc.dram_tensor("constb", [P, 24 * FQ], BF,
                              kind="ExternalInput").ap()
    out_d = nc.dram_tensor("out", [P, B], F32, kind="ExternalOutput").ap()
    import os
    dbg_n = int(os.environ.get("ARAP_DBG", "0"))
    dbg_d = (nc.dram_tensor("dbg", [P, dbg_n * BQ * FQ], BF,
                            kind="ExternalOutput").ap() if dbg_n else None)

    with tile.TileContext(nc) as tc, ExitStack() as ctx:
        cpool = ctx.enter_context(tc.tile_pool(name="consts", bufs=1))
        ppool = ctx.enter_context(tc.tile_pool(name="pred", bufs=2))
        wpool = ctx.enter_context(tc.tile_pool(name="work", bufs=96))

        cb = cpool.tile([P, 24 * FQ], BF)
        nc.sync.dma_start(cb[:, :], constb_d[:, :])
        outacc = cpool.tile([P, B], F32)
        bias_sinl = cpool.tile([P, 1], F32)
        nc.gpsimd.memset(bias_sinl[:, :], C_SINL)

        vec = nc.vector
        act = nc.scalar

        def crow3(r):
            """bf16 const row r as [P, 3, BQ, FQ] (i- and batch-bcast)."""
            a = cb[:, r * FQ:(r + 1) * FQ]
            return bass.AP(a.tensor, a.offset,
                           [list(a.ap[0]), [0, 3], [0, BQ], list(a.ap[1])])

        def wrow6():
            """wk rows 18..23 as [P, 6, BQ, FQ]."""
            a = cb[:, 18 * FQ:24 * FQ]
            return bass.AP(a.tensor, a.offset,
                           [list(a.ap[0]), [FQ, 6], [0, BQ], [1, FQ]])

        r_wp = lambda k: crow3(k)
        r_wz = lambda k: crow3(6 + k)
        r_tz = lambda k: crow3(12 + k)

        def tt(op, out, a, b):
            vec.tensor_tensor(out=out, in0=a, in1=b, op=op)

        def wt(name, dt=BF, n=1, tag=None, bufs=None):
            if tag is None:
                tag = {(BF, 1): "sg", (F32, 1): "sf", (BF, 2): "p2k",
                       (BF, 3): "t3", (BF, 6): "s6", (BF, 9): "pk9"}[
                           (dt, n)]
            if bufs is None:
                bufs = {"sg": 19, "sf": 8, "p2k": 8, "t3": 7, "s6": 4,
                        "pk9": 3, "x2": 18, "q4": 8}[tag]
            return wpool.tile([P, n * FD], dt, tag=tag, name=name,
                              uniquify=True, bufs=bufs)

        def xt(name):
            """long-lived per-pass single (2 passes in flight)."""
            return wt(name, BF, 1, tag="x2", bufs=18)

        def ent(t, s=0):
            a = t[:, :]
            return bass.AP(a.tensor, a.offset + s * FD,
                           [list(a.ap[0]), [FQ, BQ], [1, FQ]])

        def tri(t, s=0, stride=FD):
            a = t[:, :]
            return bass.AP(a.tensor, a.offset + s * FD,
                           [list(a.ap[0]), [stride, 3], [FQ, BQ], [1, FQ]])

        def pair(t):
            a = t[:, :]
            return bass.AP(a.tensor, a.offset,
                           [list(a.ap[0]), [FD, 2], [FQ, BQ], [1, FQ]])

        def six(t):
            a = t[:, :]
            return bass.AP(a.tensor, a.offset,
                           [list(a.ap[0]), [FD, 6], [FQ, BQ], [1, FQ]])

        def bc(x, n):
            """broadcast a [P, BQ, FQ] view over n."""
            return bass.AP(x.tensor, x.offset,
                           [list(x.ap[0]), [0, n]] +
                           [list(d) for d in x.ap[1:]])

        SYM = {(0, 0): 0, (1, 1): 1, (2, 2): 2,
               (0, 1): 3, (1, 0): 3, (0, 2): 4, (2, 0): 4,
               (1, 2): 5, (2, 1): 5}

        def build_pass(qb):
            s = SimpleNamespace(qb=qb)

            def S0a():
                """DMA windows, e_k, A build; issue sqA square."""
                s.pq = [ppool.tile([P, BQ * 3 * GWD], F32, tag=f"pq{g}",
                                   uniquify=True, bufs=1,
                                   name=f"pq{g}_{qb}")
                        for g in range(G)]
                span = BQ * 3 * GWD
                for g in (1, 0, 2):
                    off = (g * B + qb * BQ) * 3 * GWD
                    nc.sync.dma_start(s.pq[g][:, :],
                                      predl_d[:, off:off + span])

                def qv3(w):
                    g, slot = win_map[w]
                    a = s.pq[g][:, :]
                    return bass.AP(a.tensor, a.offset + slot,
                                   [list(a.ap[0]), [GWD, 3],
                                    [3 * GWD, BQ], [1, FQ]])

                s.Et = wpool.tile([P, 18 * FD], BF, tag="E", uniquify=True,
                                  bufs=2, name=f"E{qb}")
                s.eT = lambda k: tri(s.Et, 3 * k)
                for k in (2, 3, 0, 1, 4, 5):
                    tt(AL.subtract, s.eT(k), qv3(k + 1), qv3(0))

                H0 = wt(f"H0_{qb}", BF, 3)
                H5 = wt(f"H5_{qb}", BF, 3)
                tt(AL.mult, tri(H0), s.eT(0), r_wp(0))
                tt(AL.mult, tri(H5), s.eT(5), r_wp(5))
                s.Ap = wpool.tile([P, 9 * FD], BF, tag="A", uniquify=True,
                                  bufs=2, name=f"A{qb}")
                t3 = wt(f"t3a_{qb}", BF, 3)
                col = lambda j: tri(s.Ap, j, stride=3 * FD)
                tt(AL.mult, col(0), s.eT(4), r_wp(4))
                tt(AL.add, col(0), col(0), tri(H5))
                tt(AL.subtract, col(0), col(0), tri(H0))
                tt(AL.mult, tri(t3), s.eT(1), r_wp(1))
                tt(AL.subtract, col(0), col(0), tri(t3))
                tt(AL.mult, col(1), s.eT(3), r_wp(3))
                tt(AL.add, col(1), col(1), tri(H5))
                tt(AL.subtract, col(1), col(1), tri(H0))
                tt(AL.mult, tri(t3), s.eT(2), r_wp(2))
                tt(AL.subtract, col(1), col(1), tri(t3))
                tt(AL.mult, col(2), s.eT(0), r_wz(0))
                for k in range(1, K):
                    tt(AL.mult, tri(t3), s.eT(k), r_wz(k))
                    tt(AL.add, col(2), col(2), tri(t3))
                s.aE = lambda i, j: ent(s.Ap, i * 3 + j)
                s.sqA = wt(f"sqA_{qb}", BF, 9)
                act.square(s.sqA[:, :], s.Ap[:, :])

            def S1():
                """C = A^T A, detA; issue dsg sign + offdiag squares."""
                aE = s.aE
                s.Cp = wpool.tile([P, 6 * FD], BF, tag="C", uniquify=True,
                                  bufs=2, name=f"C{qb}")
                tt(AL.add, tri(s.Cp, 0), tri(s.sqA, 0), tri(s.sqA, 3))
                tt(AL.add, tri(s.Cp, 0), tri(s.Cp, 0), tri(s.sqA, 6))
                tmpb = wt(f"tmpc_{qb}")
                t3c = wt(f"t3c_{qb}", BF, 3)
                acol_ = lambda c: tri(s.Ap, c, stride=3 * FD)
                for i_s, (a, b) in enumerate(((0, 1), (0, 2), (1, 2))):
                    dst = ent(s.Cp, 3 + i_s)
                    tt(AL.mult, tri(t3c), acol_(a), acol_(b))
                    tt(AL.add, dst, ent(t3c, 0), ent(t3c, 1))
                    tt(AL.add, dst, dst, ent(t3c, 2))
                s.cE = lambda i_s: ent(s.Cp, i_s)
                u0, u1, u2 = wt(f"u0_{qb}"), wt(f"u1_{qb}"), wt(f"u2_{qb}")
                detA = wt(f"detA_{qb}")
                tt(AL.mult, ent(u0), aE(1, 1), aE(2, 2))
                tt(AL.mult, ent(tmpb), aE(2, 1), aE(1, 2))
                tt(AL.subtract, ent(u0), ent(u0), ent(tmpb))
                tt(AL.mult, ent(u1), aE(0, 1), aE(2, 2))
                tt(AL.mult, ent(tmpb), aE(2, 1), aE(0, 2))
                tt(AL.subtract, ent(u1), ent(u1), ent(tmpb))
                tt(AL.mult, ent(u2), aE(0, 1), aE(1, 2))
                tt(AL.mult, ent(tmpb), aE(1, 1), aE(0, 2))
                tt(AL.subtract, ent(u2), ent(u2), ent(tmpb))
                tt(AL.mult, ent(detA), aE(0, 0), ent(u0))
                tt(AL.mult, ent(tmpb), aE(1, 0), ent(u1))
                tt(AL.subtract, ent(detA), ent(detA), ent(tmpb))
                tt(AL.mult, ent(tmpb), aE(2, 0), ent(u2))
                tt(AL.add, ent(detA), ent(detA), ent(tmpb))
                s.detA = detA
                s.sqb3 = wpool.tile([P, 3 * FD], BF, tag="sqb",
                                    uniquify=True, bufs=2, name=f"sqb{qb}")
                act.square(s.sqb3[:, :], s.Cp[:, 3 * FD:6 * FD])

            def S2():
                """p1, tr, qm, b3, cross-products; issue sb3 square."""
                s.sq01, s.sq02, s.sq12 = (ent(s.sqb3, 0), ent(s.sqb3, 1),
                                          ent(s.sqb3, 2))
                s.p1 = xt(f"p1_{qb}")
                tt(AL.add, ent(s.p1), s.sq01, s.sq02)
                tt(AL.add, ent(s.p1), ent(s.p1), s.sq12)
                s.trb = xt(f"trb_{qb}")
                tt(AL.add, ent(s.trb), s.cE(0), s.cE(1))
                tt(AL.add, ent(s.trb), ent(s.trb), s.cE(2))
                s.qm = xt(f"qm_{qb}")
                act.mul(s.qm[:, :], s.trb[:, :], 1.0 / 3.0)
                s.b3 = wt(f"b3_{qb}", BF, 3)
                tt(AL.subtract, tri(s.b3), tri(s.Cp, 0), bc(ent(s.qm), 3))
                s.cp01 = xt(f"cp01_{qb}")
                s.cp02 = xt(f"cp02_{qb}")
                s.cp12 = xt(f"cp12_{qb}")
                tt(AL.mult, ent(s.cp01), s.cE(4), s.cE(5))
                tt(AL.mult, ent(s.cp02), s.cE(3), s.cE(5))
                tt(AL.mult, ent(s.cp12), s.cE(3), s.cE(4))
                s.sb3 = wt(f"sb3_{qb}", BF, 3)
                act.square(s.sb3[:, :], s.b3[:, :])

            def S3():
                """p2; issue ln/exp block; detC as filler."""
                s.p2 = wt(f"p2_{qb}")
                tt(AL.add, ent(s.p2), ent(s.sb3, 0), ent(s.sb3, 1))
                tt(AL.add, ent(s.p2), ent(s.p2), ent(s.sb3, 2))
                vec.scalar_tensor_tensor(out=ent(s.p2), in0=ent(s.p1),
                                         scalar=2.0, in1=ent(s.p2),
                                         op0=AL.mult, op1=AL.add)
                vec.tensor_scalar_max(out=s.p2[:, :], in0=s.p2[:, :],
                                      scalar1=1e-12)
                s.lnp6 = wt(f"lnp6_{qb}", F32)
                act.activation(s.lnp6[:, :], s.p2[:, :], AF.Ln,
                               scale=4.0 / 6.0)
                s.two_p = wt(f"two_p_{qb}", F32)
                act.activation(s.two_p[:, :], s.lnp6[:, :], AF.Exp,
                               scale=0.5)
                s.pinv8 = wt(f"pinv8_{qb}", F32)
                act.activation(s.pinv8[:, :], s.lnp6[:, :], AF.Exp,
                               scale=-1.5)
                s.two_pb = wt(f"two_pb_{qb}")
                act.copy(s.two_pb[:, :], s.two_p[:, :])
                # detC (DVE filler, independent of the ACT chain)
                b0, b1, b2 = ent(s.b3, 0), ent(s.b3, 1), ent(s.b3, 2)
                tmpb = wt(f"tmpd_{qb}")
                ub0, ub1, ub2 = (wt(f"ub0_{qb}"), wt(f"ub1_{qb}"),
                                 wt(f"ub2_{qb}"))
                tt(AL.mult, ent(ub0), b1, b2)
                tt(AL.subtract, ent(ub0), ent(ub0), s.sq12)
                tt(AL.mult, ent(ub1), s.cE(3), b2)
                tt(AL.subtract, ent(ub1), ent(ub1), ent(s.cp01))
                tt(AL.mult, ent(ub2), b1, s.cE(4))
                tt(AL.subtract, ent(ub2), ent(s.cp02), ent(ub2))
                s.detC = wt(f"detC_{qb}")
                tt(AL.mult, ent(s.detC), b0, ent(ub0))
                tt(AL.mult, ent(tmpb), s.cE(3), ent(ub1))
                tt(AL.subtract, ent(s.detC), ent(s.detC), ent(tmpb))
                tt(AL.mult, ent(tmpb), s.cE(4), ent(ub2))
                tt(AL.add, ent(s.detC), ent(s.detC), ent(tmpb))

            def S4():
                """r; issue r2/lnomr/eh."""
                s.r = wt(f"r_{qb}", F32)
                vec.scalar_tensor_tensor(out=ent(s.r), in0=ent(s.detC),
                                         scalar=4.0, in1=ent(s.pinv8),
                                         op0=AL.mult, op1=AL.mult)
                vec.tensor_scalar(out=s.r[:, :], in0=s.r[:, :],
                                  scalar1=RCLAMP, scalar2=-RCLAMP,
                                  op0=AL.min, op1=AL.max)
                r2 = wt(f"r2_{qb}", F32)
                act.square(r2[:, :], s.r[:, :])
                lnomr = wt(f"lnomr_{qb}", F32)
                act.activation(lnomr[:, :], r2[:, :], AF.Ln, bias=1.0,
                               scale=-1.0)
                s.eh = wt(f"eh_{qb}", F32)
                act.activation(s.eh[:, :], lnomr[:, :], AF.Exp, scale=-0.5)

            def S5s():
                s.s_ = wt(f"s__{qb}", F32)
                tt(AL.mult, ent(s.s_), ent(s.r), ent(s.eh))

            def S5at():
                s.at = wt(f"at_{qb}", F32)
                act.activation(s.at[:, :], s.s_[:, :], AF.Arctan)

            def S5sin():
                s.sinL = wt(f"sinL_{qb}")
                act.activation(s.sinL[:, :], s.at[:, :], AF.Sin,
                               bias=bias_sinl[:, :], scale=-1.0 / 3.0)
                s.sinM = wt(f"sinM_{qb}")
                act.activation(s.sinM[:, :], s.at[:, :], AF.Sin,
                               scale=-1.0 / 3.0)

            def S6():
                """eigenvalues, gaps, clamps; issue the g/recip ACT block."""
                tmpb = wt(f"tmpe_{qb}")
                s.lam3, s.lam1 = xt(f"lam3_{qb}"), xt(f"lam1_{qb}")
                lam2 = wt(f"lam2_{qb}")
                tt(AL.mult, ent(tmpb), ent(s.two_pb), ent(s.sinL))
                tt(AL.add, ent(s.lam3), ent(s.qm), ent(tmpb))
                tt(AL.mult, ent(tmpb), ent(s.two_pb), ent(s.sinM))
                tt(AL.add, ent(lam2), ent(s.qm), ent(tmpb))
                tt(AL.subtract, ent(tmpb), ent(s.trb), ent(s.lam3))
                tt(AL.subtract, ent(s.lam1), ent(tmpb), ent(lam2))
                tt(AL.subtract, ent(tmpb), ent(s.sinL), ent(s.sinM))
                # d-quad = [d21, d31, ssum(later), d32]
                s.dq = wt(f"dq_{qb}", BF, 4, tag="q4", bufs=8)
                tt(AL.subtract, ent(s.dq, 0), ent(lam2), ent(s.lam1))
                tt(AL.subtract, ent(s.dq, 1), ent(s.lam3), ent(s.lam1))
                tt(AL.mult, ent(s.dq, 3), ent(s.two_pb), ent(tmpb))
                # l-pair = [max(lam2, clip), max(lam3, clip)]
                s.lp = wt(f"lp_{qb}", BF, 2)
                vec.tensor_scalar_max(out=s.lp[:, 0:FD], in0=lam2[:, :],
                                      scalar1=CLIPV)
                vec.tensor_scalar_max(out=s.lp[:, FD:2 * FD],
                                      in0=s.lam3[:, :], scalar1=CLIPV)
                # eps-quad = [l3q, l3q, l3c, l3q] (x^2-scales of dq slots)
                s.epsq = wt(f"epsq_{qb}", BF, 4, tag="q4", bufs=8)
                vec.tensor_copy(s.epsq[:, 2 * FD:3 * FD],
                                s.lp[:, FD:2 * FD])
                # ACT block (all natural_log_exp set): g-pair, l3^4
                lnl = wt(f"lnl_{qb}", BF, 2)
                act.activation(lnl[:, :], s.lp[:, :], AF.Ln)
                s.gP = wt(f"gP_{qb}", BF, 2)
                act.activation(s.gP[:, :], lnl[:, :], AF.Exp, scale=-0.5)
                l3sq = wt(f"l3sq_{qb}")
                act.square(l3sq[:, :], s.lp[:, FD:2 * FD])
                s.l3q = wt(f"l3q_{qb}")
                act.square(s.l3q[:, :], l3sq[:, :])

            def S7a():
                """[needs gP, l3q] ssum/q23/eps-quad; issue square(dq)."""
                s.sq3 = wt(f"sq3_{qb}")
                tt(AL.mult, ent(s.sq3), ent(s.lp, 1), ent(s.gP, 1))
                tmps = wt(f"tmps_{qb}")
                tt(AL.mult, ent(tmps), ent(s.lp, 0), ent(s.gP, 0))
                tt(AL.add, ent(s.dq, 2), ent(tmps), ent(s.sq3))
                s.q23 = wt(f"q23_{qb}")
                tt(AL.mult, ent(s.q23), ent(s.gP, 0), ent(s.gP, 1))
                s.dsg = xt(f"dsg_{qb}")
                act.sign(s.dsg[:, :], s.detA[:, :])
                s.selb = xt(f"selb_{qb}")
                act.activation(s.selb[:, :], s.dsg[:, :], AF.Copy, bias=0.5,
                               scale=0.5)
                lq = s.l3q[:, :]
                lqb = bass.AP(lq.tensor, lq.offset,
                              [list(lq.ap[0]), [0, 2], [1, FD]])
                dst01 = s.epsq[:, 0:2 * FD]
                vec.tensor_copy(bass.AP(dst01.tensor, dst01.offset,
                                        [list(dst01.ap[0]), [FD, 2],
                                         [1, FD]]), lqb)
                vec.tensor_copy(s.epsq[:, 3 * FD:4 * FD], lq)
                s.sqq = wt(f"sqq_{qb}", BF, 4, tag="q4", bufs=8)
                act.square(s.sqq[:, :], s.dq[:, :])

            def S7a2():
                """[needs sqq] eps-add + floor; issue ln/exp quad."""
                vec.scalar_tensor_tensor(out=s.sqq[:, :],
                                         in0=s.epsq[:, :],
                                         scalar=1e-12, in1=s.sqq[:, :],
                                         op0=AL.mult, op1=AL.add)
                vec.tensor_scalar_max(out=s.sqq[:, :], in0=s.sqq[:, :],
                                      scalar1=1e-30)
                act.activation(s.sqq[:, :], s.sqq[:, :], AF.Ln)
                s.eiq = wt(f"eiq_{qb}", BF, 4, tag="q4", bufs=8)
                act.activation(s.eiq[:, :], s.sqq[:, :], AF.Exp,
                               scale=-1.0)

            def S7b():
                """gam's, T2, W2, Y, R, energy residuals (big DVE block)."""
                # N1/Md/T2 first: независимы of the reciprocal quad, so the
                # DVE has work while ACT finishes the ln/exp for iq.
                N1p = wt(f"N1p_{qb}", BF, 6)
                tt(AL.subtract, tri(N1p, 0), tri(s.Cp, 0),
                   bc(ent(s.lam1), 3))
                vec.tensor_copy(N1p[:, 3 * FD:6 * FD],
                                s.Cp[:, 3 * FD:6 * FD])
                Md3 = wt(f"Md3_{qb}", BF, 3)
                tt(AL.subtract, tri(Md3), tri(s.Cp, 0), bc(ent(s.lam3), 3))
                T2p = wt(f"T2p_{qb}", BF, 6)
                tt(AL.mult, tri(T2p, 0), tri(N1p, 0), tri(Md3))
                tt(AL.add, ent(T2p, 0), ent(T2p, 0), s.sq01)
                tt(AL.add, ent(T2p, 0), ent(T2p, 0), s.sq02)
                tt(AL.add, ent(T2p, 1), ent(T2p, 1), s.sq01)
                tt(AL.add, ent(T2p, 1), ent(T2p, 1), s.sq12)
                tt(AL.add, ent(T2p, 2), ent(T2p, 2), s.sq02)
                tt(AL.add, ent(T2p, 2), ent(T2p, 2), s.sq12)
                tq = wt(f"tq_{qb}")
                for (slot, a, mslot, cslot, cpx) in (
                        (3, 0, 1, 3, s.cp01), (4, 0, 2, 4, s.cp02),
                        (5, 1, 2, 5, s.cp12)):
                    tt(AL.add, ent(tq), ent(N1p, a), ent(Md3, mslot))
                    tt(AL.mult, ent(T2p, slot), s.cE(cslot), ent(tq))
                    tt(AL.add, ent(T2p, slot), ent(T2p, slot), ent(cpx))
                # i-quad = [1/d21, 1/d31, 1/ssum, 1/d32]
                iq = s.eiq
                vec.tensor_tensor(out=iq[:, :], in0=s.dq[:, :],
                                  in1=iq[:, :], op=AL.mult)
                s.p2131 = wt(f"p2131_{qb}")
                tt(AL.mult, ent(s.p2131), ent(iq, 0), ent(iq, 1))
                c3, c4 = wt(f"c3_{qb}"), wt(f"c4_{qb}")
                t1 = wt(f"t1_{qb}")
                # c3+ = -(d21/ssum + sq3) * q23 * i21*i31
                tt(AL.mult, ent(t1), ent(s.dq, 0), ent(iq, 2))
                tt(AL.add, ent(t1), ent(t1), ent(s.sq3))
                tt(AL.mult, ent(t1), ent(t1), ent(s.q23))
                c3p = wt(f"c3p_{qb}")
                vec.scalar_tensor_tensor(out=ent(c3p), in0=ent(t1),
                                         scalar=-1.0, in1=ent(s.p2131),
                                         op0=AL.mult, op1=AL.mult)
                # c3- = -(g3*d21 + g2*d31) / (d21*d31*d32)
                t2 = wt(f"t2_{qb}")
                tt(AL.mult, ent(t2), ent(s.gP, 1), ent(s.dq, 0))
                tt(AL.mult, ent(t1), ent(s.gP, 0), ent(s.dq, 1))
                tt(AL.add, ent(t2), ent(t2), ent(t1))
                tt(AL.mult, ent(t2), ent(t2), ent(iq, 3))
                c3m = wt(f"c3m_{qb}")
                vec.scalar_tensor_tensor(out=ent(c3m), in0=ent(t2),
                                         scalar=-1.0, in1=ent(s.p2131),
                                         op0=AL.mult, op1=AL.mult)
                # blend on sign: c3 = c3m + (dsg+1)/2 * (c3p - c3m)
                selb = s.selb
                tt(AL.subtract, ent(t1), ent(c3p), ent(c3m))
                tt(AL.mult, ent(t1), ent(selb), ent(t1))
                tt(AL.add, ent(c3), ent(c3m), ent(t1))
                # c4 = dsg * g3 * i31
                tt(AL.mult, ent(c4), ent(s.gP, 1), ent(iq, 1))
                tt(AL.mult, ent(c4), ent(s.dsg), ent(c4))
                # W2 = c3*T2 + c4*N1 (in place on T2p)
                t6 = wt(f"t6_{qb}", BF, 6)
                tt(AL.mult, six(t6), six(N1p), bc(ent(c4), 6))
                tt(AL.mult, six(T2p), six(T2p), bc(ent(c3), 6))
                tt(AL.add, six(T2p), six(T2p), six(t6))
                w2 = lambda cc, j: bc(ent(T2p, SYM[(cc, j)]), 3)
                s.dbg_W2, s.dbg_c3, s.dbg_c4 = T2p, c3, c4
                # Y = A @ W2
                Yp = wt(f"Yp_{qb}", BF, 9)
                t3 = wt(f"t3b_{qb}", BF, 3)
                acol = lambda cc: tri(s.Ap, cc, stride=3 * FD)
                ycol = lambda j: tri(Yp, j, stride=3 * FD)
                for j in range(3):
                    tt(AL.mult, ycol(j), acol(0), w2(0, j))
                    for cc in (1, 2):
                        tt(AL.mult, tri(t3), acol(cc), w2(cc, j))
                        tt(AL.add, ycol(j), ycol(j), tri(t3))
                # R = Y + cof(Y)
                s.dbg_Yp = Yp
                Rp = wt(f"Rp_{qb}", BF, 9)
                s.dbg_Rp = Rp
                yE = lambda i, j: ent(Yp, i * 3 + j)
                cf = wt(f"cf_{qb}")
                tmpb = wt(f"tmpf_{qb}")
                for i in range(3):
                    for j in range(3):
                        i1, i2 = (i + 1) % 3, (i + 2) % 3
                        j1, j2 = (j + 1) % 3, (j + 2) % 3
                        tt(AL.mult, ent(cf), yE(i1, j1), yE(i2, j2))
                        tt(AL.mult, ent(tmpb), yE(i1, j2), yE(i2, j1))
                        tt(AL.subtract, ent(cf), ent(cf), ent(tmpb))
                        tt(AL.add, ent(Rp, i * 3 + j), yE(i, j), ent(cf))
                # energy residuals (software-pipelined with ACT squares)
                rcol = lambda j: tri(Rp, j, stride=3 * FD)
                Rpm3 = wt(f"Rpm3_{qb}", BF, 3)
                tt(AL.add, tri(Rpm3), rcol(0), rcol(1))
                Z3 = wt(f"Z3_{qb}", BF, 3)
                dfc = [wt(f"dfc{i}_{qb}", BF, 3) for i in (0, 1)]
                sqd = [wt(f"sqd{i}_{qb}", BF, 3) for i in (0, 1)]
                s.ns6 = wt(f"ns6_{qb}", BF, 6)
                combos = ((AL.add, tri(Rpm3)), (AL.add, rcol(0)),
                          (AL.add, rcol(1)), (AL.subtract, rcol(1)),
                          (AL.subtract, rcol(0)), (AL.subtract, tri(Rpm3)))

                def emit_dfc(k):
                    d = dfc[k % 2]
                    tt(AL.mult, tri(Z3), rcol(2), r_tz(k))
                    tt(AL.subtract, tri(d), s.eT(k), tri(Z3))
                    op, cv = combos[k]
                    tt(op, tri(d), tri(d), cv)
                    act.square(sqd[k % 2][:, :], d[:, :])

                def emit_ns(k):
                    sq = sqd[k % 2]
                    tt(AL.add, ent(s.ns6, k), ent(sq, 0), ent(sq, 1))
                    tt(AL.add, ent(s.ns6, k), ent(s.ns6, k), ent(sq, 2))

                emit_dfc(0)
                for k in range(1, K):
                    emit_dfc(k)
                    emit_ns(k - 1)
                emit_ns(K - 1)

            def S8sqrt():
                if dbg_n and qb == 0:
                    FDl = FD

                    def dump(slot, t, n):
                        nc.sync.dma_start(
                            dbg_d[:, slot * FDl:(slot + n) * FDl],
                            t[:, 0:n * FDl])
                    dump(0, s.Ap, 9)
                    dump(9, s.Cp, 6)
                    dump(15, s.gP, 2)
                    dump(17, s.dbg_c3, 1)
                    dump(18, s.dbg_c4, 1)
                    dump(19, s.lam1, 1)
                    dump(20, s.lam3, 1)
                    dump(21, s.dbg_W2, 6)
                    dump(27, s.dbg_Yp, 9)
                    dump(36, s.dbg_Rp, 9)
                    dump(45, s.Et, 3)
                    dump(48, s.dsg, 1)
                    dump(49, s.ns6, 6)
                    dump(55, s.dp, 2)
                    dump(57, s.dsq, 2)
                    dump(59, s.ei, 2)
                    dump(61, s.l3q, 1)
                act.activation(s.ns6[:, :], s.ns6[:, :], AF.Sqrt)

            def S8():
                tt(AL.mult, six(s.ns6), six(s.ns6), wrow6())
                s3t = wt(f"s3_{qb}", BF, 3)
                tt(AL.add, tri(s3t), tri(s.ns6, 0), tri(s.ns6, 3))
                nrg = wt(f"nrg_{qb}")
                tt(AL.add, ent(nrg), ent(s3t, 0), ent(s3t, 1))
                tt(AL.add, ent(nrg), ent(nrg), ent(s3t, 2))
                vec.tensor_scalar_min(out=nrg[:, :], in0=nrg[:, :],
                                      scalar1=1.0)
                vec.tensor_reduce(out=outacc[:, qb * BQ:(qb + 1) * BQ],
                                  in_=ent(nrg), axis=mybir.AxisListType.X,
                                  op=AL.add)

            return [S0a, S1, S2, S3, S4, S5s, S5at, S5sin, S6, S7a,
                    S7a2, S7b, S8sqrt, S8]

        pending = None
        for q0 in range(0, NQ, 2):
            segsA = build_pass(q0)
            segsB = build_pass(q0 + 1)
            segsA[0]()
            segsB[0]()
            if pending is not None:
                pending[0]()
                pending[1]()
            for sa, sb in zip(segsA[1:-1], segsB[1:-1]):
                sa()
                sb()
            pending = (segsA[-1], segsB[-1])
        pending[0]()
        pending[1]()

        nc.sync.dma_start(out_d[:, :], outacc[:, :])

    nc.compile()
    return nc


def _get_nc(wingeo):
    if wingeo not in _nc_cache:
        _pin_act_tables()
        _nc_cache[wingeo] = _build_nc(wingeo)
    return _nc_cache[wingeo]


# ---------------------------------------------------------------------------
# Entry point
# ---------------------------------------------------------------------------

def _install_ntff_shim():
    """Provide antenv.axon_hooks (missing in this image) so
    run_bass_kernel_spmd(trace=True) can reach the NTFF profiler in
    libaxon_pjrt.so."""
    import types

    try:
        import antenv.axon_hooks  # noqa: F401
        return True
    except ImportError:
        pass
    try:
        import antenv
        from trn_agent_boot.trn_boot import _ntff_profile_via_ctypes
    except ImportError:
        return False
    mod = types.ModuleType("antenv.axon_hooks")
    state = {"hook": None}
    mod.set_axon_ntff_profile_hook = lambda h: state.__setitem__("hook", h)
    mod.get_axon_ntff_profile_hook = lambda: state["hook"]
    sys.modules["antenv.axon_hooks"] = mod
    antenv.axon_hooks = mod
    try:
        hook = _ntff_profile_via_ctypes("/opt/axon/libaxon_pjrt.so")
    except OSError:
        hook = None
    if hook is not None:
        mod.set_axon_ntff_profile_hook(hook)
    return hook is not None


def kernel(**inputs) -> np.ndarray:
    pred = np.asarray(inputs["prediction"], np.float32)
    adj_idx = np.asarray(inputs["adj_list_indices"])
    adj_w = np.asarray(inputs["adj_list_weights"], np.float32)
    tev_T = np.asarray(inputs["template_edge_vectors_T"], np.float32)

    wk, tzk = _build_offset_classes(adj_idx, adj_w, tev_T)
    in_maps, wingeo = _host_prepare(pred, wk, tzk)

    nc = _get_nc(wingeo)
    import os
    trace = bool(int(os.environ.get("ARAP_TRACE", "0")))
    if trace:
        trace = _install_ntff_shim()
    try:
        res = run_bass_kernel_spmd(nc, in_maps, core_ids=list(range(NCORES)),
                                   trace=trace)
    except Exception:
        if not trace:
            raise
        res = run_bass_kernel_spmd(nc, in_maps, core_ids=list(range(NCORES)),
                                   trace=False)
    kernel._last_exec_ns = res.exec_time_ns
    kernel._last_results = res

    total = np.zeros(B, np.float64)
    for c in range(NCORES):
        total += res.results[c]["out"].astype(np.float64).sum(axis=0)
    return (total / NV).astype(np.float32)


kernel._last_exec_ns = None


# revision 30
# speedup vs baseline: 1.0006x; 1.0006x over previous
"""ARAP loss kernel for Trainium2 (8 NeuronCores, SPMD over the vertex axis).

Problem: nn_ArapLoss — per-vertex 6-neighbor gather on a 316x316 grid mesh,
3x3 polar decomposition (closed-form symmetric eigenanalysis) per vertex,
cotan-weighted edge-residual energy, clamped mean over vertices.

Strategy (v3 — vector-engine lean, pair-pipelined)
--------------------------------------------------
- Shard the vertex axis N=99856 across 8 cores (12482 each, padded to
  12544 = 128*98).  Grid adjacency reduces to K=6 constant index offsets
  {+-1, +-316, +-317}; the host materializes shifted windows of
  `prediction` so the device does no gather.
- Edge vectors e_k = q_{n+o_k} - p_n are computed ONCE in f32 and stored
  bf16; everything downstream runs in bf16 (DVE 2x mode).
- The template-edge xy components are EXACTLY {0,+-1} per offset class
  (regular grid), so A = sum_k e_k (stab w_k t_k)^T collapses to signed
  sums plus one weighted z-column, and the rotated-template residual
  e_k - R t_k collapses to (e_k - tz_k R[:,2]) -+ R-column combos.
- R from a SINGLE 3x3 product:  R = Y + cof(Y),  Y = A (g2 P2 + d g3 P3).
  cof(u2 v2' + d u3 v3') = d^2 u1 v1' = u1 v1', so the smallest-eigenvalue
  component needs no division by s1 and no second product / sign fixup.
- Safe reciprocals as x/(x^2 + eps) — no Abs/Sign, sign rides the x.
- Passes are emitted in PAIRS, interleaved at every ACT-dependency
  boundary, so the in-order DVE queue always has independent work while
  the scalar engine walks its serial ln/exp/arctan/sin chain.  The trig
  and sqrt activations of the two passes share one table-load block.
- Output: per-core partial sums [128, B]; host reduces and divides by N.
"""
import sys

for _p in ("/opt/trn_rl_repo", "/opt/trn_rl_repo/concourse", "/opt/pypackages"):
    if _p not in sys.path:
        sys.path.insert(0, _p)

from types import SimpleNamespace

from contextlib import ExitStack

import ml_dtypes
import numpy as np

import concourse.bass as bass
import concourse.tile as tile
from concourse import bacc, mybir
from concourse.bass_utils import run_bass_kernel_spmd

F32 = mybir.dt.float32
BF = mybir.dt.bfloat16
AL = mybir.AluOpType
AF = mybir.ActivationFunctionType

# ---- problem geometry (hardcoded per spec) --------------------------------
B = 16
NV = 99856
NCORES = 8
P = 128
NC_V = NV // NCORES            # 12482 real vertices per core
FQ = 98                        # free-dim vertices per partition
VP = P * FQ                    # 12544 padded vertices per core
BQ = 4                         # batch elements per pass
NQ = B // BQ
K = 6
CLIPV = 1e-12                  # eigenvalue clamp (unscaled units)
C_SINL = float(2.0 * np.pi / 3.0)
RCLAMP = 1.0 - 1e-6
OFFS = (-317, -316, -1, 1, 316, 317)
# xy components of template edges per offset class (exact on the grid)
CX = (-1, -1, 0, 0, 1, 1)
CY = (-1, 0, -1, 1, 0, 1)

_nc_cache = {}


def _pin_act_tables():
    """Shrink the cached activation-table membership map so the compiler
    assigns Ln/Exp/Square/Sign/Abs/Copy to the one combined set that
    physically contains them all (natural_log_exp_and_others), Sin to
    trig_and_small, Arctan to sigmoid_and_others.  Every set we leave a
    function in really does contain it, so the emitted table loads stay
    valid — this only stops the compiler from ping-ponging between the
    ln-only and exp-only sets.  Best effort: on any surprise, leave the
    tables untouched (costs extra table loads, still correct)."""
    try:
        from concourse.hw_specs import get_activation_tables
        tabs = None
        for arch in ("gen3", "TRN2"):
            try:
                tabs = get_activation_tables(arch)
                break
            except Exception:
                continue
        if tabs is None:
            return
        combined = tabs.get("natural_log_exp_and_others")
        shared = {AF.Ln, AF.Exp, AF.Square, AF.Sign, AF.Abs, AF.Copy,
                  AF.Identity}
        if combined is None or not (shared <= combined):
            return
        if AF.Sin not in tabs.get("trig_and_small", set()):
            return
        for name, fns in tabs.items():
            if name == "natural_log_exp_and_others":
                continue
            fns -= shared
            if name != "trig_and_small":
                fns.discard(AF.Sin)
    except Exception:
        pass


# ---------------------------------------------------------------------------
# Host-side preprocessing
# ---------------------------------------------------------------------------

def _build_offset_classes(adj_idx, adj_w, tev_T):
    """(N,D) adjacency -> per-offset-class weights wk (K,N) and template
    edge z-components tzk (K,N).  Asserts the grid structure this kernel
    hardcodes (xy components == CX/CY per class)."""
    N, D = adj_idx.shape
    ar = np.arange(N, dtype=np.int64)
    real = (adj_idx > 0) | (np.arange(D)[None, :] == 0)
    delta = np.asarray(adj_idx, np.int64) - ar[:, None]
    offs = np.unique(delta[real])
    assert tuple(int(o) for o in offs) == OFFS, f"unexpected offsets {offs}"
    wk = np.zeros((K, N), np.float32)
    tzk = np.zeros((K, N), np.float32)
    for k, o in enumerate(OFFS):
        sel = real & (delta == o)
        n_id, d_id = np.nonzero(sel)
        wk[k, n_id] = adj_w[n_id, d_id]
        tzk[k, n_id] = tev_T[n_id, 2, d_id]
    return wk, tzk


def _group_offsets(gap=8):
    """Group [0]+OFFS into consecutive runs; returns (bases, width, win_map)
    where win_map[x] = (g, slot) for x in [0(center)] + OFFS order."""
    allo = sorted(set([0] + list(OFFS)))
    groups = [[allo[0]]]
    for o in allo[1:]:
        if o - groups[-1][-1] <= gap:
            groups[-1].append(o)
        else:
            groups.append([o])
    bases = [g[0] for g in groups]
    width = FQ + max(g[-1] - g[0] for g in groups) + 1
    lut = {}
    for gi, g in enumerate(groups):
        for o in g:
            lut[o] = (gi, o - g[0])
    win_map = [lut[0]] + [lut[o] for o in OFFS]
    return bases, width, tuple(win_map)


def _host_prepare(pred, wk, tzk):
    """Build per-core input maps: predl [P, G*B*3*GWD] f32 (group-major so
    each pass loads G contiguous chunks) and constb [P, 24*FQ] bf16
    (rows: wp(6), wz(6), tz(6), wk(6))."""
    bases, GWD, win_map = _group_offsets()
    G = len(bases)
    H = max(max(abs(o) for o in OFFS), 1)
    padlen = NV + 2 * H + (VP - NC_V) + GWD
    padG = np.zeros((B, 3, padlen), np.float32)
    padG[:, :, H:H + NV] = pred

    wp = wk                              # (K, N) — no stab scaling: R is
    wz = wp * tzk                        # scale-invariant; bf16 ranges stay sane
    CG = np.concatenate([wp, wz, tzk, wk], axis=0)   # (24, N)

    in_maps = []
    pidx = (np.arange(P)[:, None] * FQ + np.arange(GWD)[None, :])  # (P,GWD)
    for c in range(NCORES):
        base = c * NC_V
        wins = np.empty((G, B, 3, P, GWD), np.float32)
        for g, bg in enumerate(bases):
            idx = H + base + bg + pidx
            wins[g] = padG[:, :, idx].transpose(0, 1, 2, 3)
        predl = np.ascontiguousarray(
            wins.transpose(3, 0, 1, 2, 4)
        ).reshape(P, G * B * 3 * GWD)

        cc = np.zeros((24, VP), np.float32)
        hi = min(base + VP, NV) - base
        hi = min(hi, NC_V)                   # zero weights on padded tail
        cc[:, :hi] = CG[:, base:base + hi]
        constb = np.ascontiguousarray(
            cc.reshape(24, P, FQ).transpose(1, 0, 2)
        ).reshape(P, 24 * FQ).astype(ml_dtypes.bfloat16)

        in_maps.append({"predl": predl, "constb": constb})
    return in_maps, (G, GWD, win_map)


# ---------------------------------------------------------------------------
# Device kernel builder
# ---------------------------------------------------------------------------

def _build_nc(wingeo):
    G, GWD, win_map = wingeo
    FD = BQ * FQ

    nc = bacc.Bacc("TRN2", target_bir_lowering=False, debug=False,
                   num_devices=NCORES)

    predl_d = nc.dram_tensor("predl", [P, G * B * 3 * GWD], F32,
                             kind="ExternalInput").ap()
    constb_d = nc.dram_tensor("constb", [P, 24 * FQ], BF,
                              kind="ExternalInput").ap()
    out_d = nc.dram_tensor("out", [P, B], F32, kind="ExternalOutput").ap()
    import os
    dbg_n = int(os.environ.get("ARAP_DBG", "0"))
    dbg_d = (nc.dram_tensor("dbg", [P, dbg_n * BQ * FQ], BF,
                            kind="ExternalOutput").ap() if dbg_n else None)

    with tile.TileContext(nc) as tc, ExitStack() as ctx:
        cpool = ctx.enter_context(tc.tile_pool(name="consts", bufs=1))
        ppool = ctx.enter_context(tc.tile_pool(name="pred", bufs=2))
        wpool = ctx.enter_context(tc.tile_pool(name="work", bufs=96))

        cb = cpool.tile([P, 24 * FQ], BF)
        nc.sync.dma_start(cb[:, :], constb_d[:, :])
        outacc = cpool.tile([P, B], F32)
        bias_sinl = cpool.tile([P, 1], F32)
        nc.gpsimd.memset(bias_sinl[:, :], C_SINL)

        vec = nc.vector
        act = nc.scalar

        def crow3(r):
            """bf16 const row r as [P, 3, BQ, FQ] (i- and batch-bcast)."""
            a = cb[:, r * FQ:(r + 1) * FQ]
            return bass.AP(a.tensor, a.offset,
                           [list(a.ap[0]), [0, 3], [0, BQ], list(a.ap[1])])

        def wrow6():
            """wk rows 18..23 as [P, 6, BQ, FQ]."""
            a = cb[:, 18 * FQ:24 * FQ]
            return bass.AP(a.tensor, a.offset,
                           [list(a.ap[0]), [FQ, 6], [0, BQ], [1, FQ]])

        r_wp = lambda k: crow3(k)
        r_wz = lambda k: crow3(6 + k)
        r_tz = lambda k: crow3(12 + k)

        def tt(op, out, a, b):
            vec.tensor_tensor(out=out, in0=a, in1=b, op=op)

        def wt(name, dt=BF, n=1, tag=None, bufs=None):
            if tag is None:
                tag = {(BF, 1): "sg", (F32, 1): "sf", (BF, 2): "p2k",
                       (BF, 3): "t3", (BF, 6): "s6", (BF, 9): "pk9"}[
                           (dt, n)]
            if bufs is None:
                bufs = {"sg": 19, "sf": 8, "p2k": 8, "t3": 7, "s6": 4,
                        "pk9": 3, "x2": 18, "q4": 8}[tag]
            return wpool.tile([P, n * FD], dt, tag=tag, name=name,
                              uniquify=True, bufs=bufs)

        def xt(name):
            """long-lived per-pass single (2 passes in flight)."""
            return wt(name, BF, 1, tag="x2", bufs=18)

        def ent(t, s=0):
            a = t[:, :]
            return bass.AP(a.tensor, a.offset + s * FD,
                           [list(a.ap[0]), [FQ, BQ], [1, FQ]])

        def tri(t, s=0, stride=FD):
            a = t[:, :]
            return bass.AP(a.tensor, a.offset + s * FD,
                           [list(a.ap[0]), [stride, 3], [FQ, BQ], [1, FQ]])

        def pair(t):
            a = t[:, :]
            return bass.AP(a.tensor, a.offset,
                           [list(a.ap[0]), [FD, 2], [FQ, BQ], [1, FQ]])

        def six(t):
            a = t[:, :]
            return bass.AP(a.tensor, a.offset,
                           [list(a.ap[0]), [FD, 6], [FQ, BQ], [1, FQ]])

        def bc(x, n):
            """broadcast a [P, BQ, FQ] view over n."""
            return bass.AP(x.tensor, x.offset,
                           [list(x.ap[0]), [0, n]] +
                           [list(d) for d in x.ap[1:]])

        SYM = {(0, 0): 0, (1, 1): 1, (2, 2): 2,
               (0, 1): 3, (1, 0): 3, (0, 2): 4, (2, 0): 4,
               (1, 2): 5, (2, 1): 5}

        def build_pass(qb):
            s = SimpleNamespace(qb=qb)

            def S0a():
                """DMA windows, e_k, A build; issue sqA square."""
                s.pq = [ppool.tile([P, BQ * 3 * GWD], F32, tag=f"pq{g}",
                                   uniquify=True, bufs=1,
                                   name=f"pq{g}_{qb}")
                        for g in range(G)]
                span = BQ * 3 * GWD
                for g in (1, 0, 2):
                    off = (g * B + qb * BQ) * 3 * GWD
                    nc.sync.dma_start(s.pq[g][:, :],
                                      predl_d[:, off:off + span])

                def qv3(w):
                    g, slot = win_map[w]
                    a = s.pq[g][:, :]
                    return bass.AP(a.tensor, a.offset + slot,
                                   [list(a.ap[0]), [GWD, 3],
                                    [3 * GWD, BQ], [1, FQ]])

                s.Et = wpool.tile([P, 18 * FD], BF, tag="E", uniquify=True,
                                  bufs=2, name=f"E{qb}")
                s.eT = lambda k: tri(s.Et, 3 * k)
                for k in (2, 3, 0, 1, 4, 5):
                    tt(AL.subtract, s.eT(k), qv3(k + 1), qv3(0))

                H0 = wt(f"H0_{qb}", BF, 3)
                H5 = wt(f"H5_{qb}", BF, 3)
                tt(AL.mult, tri(H0), s.eT(0), r_wp(0))
                tt(AL.mult, tri(H5), s.eT(5), r_wp(5))
                s.Ap = wpool.tile([P, 9 * FD], BF, tag="A", uniquify=True,
                                  bufs=2, name=f"A{qb}")
                t3 = wt(f"t3a_{qb}", BF, 3)
                col = lambda j: tri(s.Ap, j, stride=3 * FD)
                tt(AL.mult, col(0), s.eT(4), r_wp(4))
                tt(AL.add, col(0), col(0), tri(H5))
                tt(AL.subtract, col(0), col(0), tri(H0))
                tt(AL.mult, tri(t3), s.eT(1), r_wp(1))
                tt(AL.subtract, col(0), col(0), tri(t3))
                tt(AL.mult, col(1), s.eT(3), r_wp(3))
                tt(AL.add, col(1), col(1), tri(H5))
                tt(AL.subtract, col(1), col(1), tri(H0))
                tt(AL.mult, tri(t3), s.eT(2), r_wp(2))
                tt(AL.subtract, col(1), col(1), tri(t3))
                tt(AL.mult, col(2), s.eT(0), r_wz(0))
                for k in range(1, K):
                    tt(AL.mult, tri(t3), s.eT(k), r_wz(k))
                    tt(AL.add, col(2), col(2), tri(t3))
                s.aE = lambda i, j: ent(s.Ap, i * 3 + j)
                s.sqA = wt(f"sqA_{qb}", BF, 9)
                act.square(s.sqA[:, :], s.Ap[:, :])

            def S1():
                """C = A^T A, detA; issue dsg sign + offdiag squares."""
                aE = s.aE
                s.Cp = wpool.tile([P, 6 * FD], BF, tag="C", uniquify=True,
                                  bufs=2, name=f"C{qb}")
                tt(AL.add, tri(s.Cp, 0), tri(s.sqA, 0), tri(s.sqA, 3))
                tt(AL.add, tri(s.Cp, 0), tri(s.Cp, 0), tri(s.sqA, 6))
                tmpb = wt(f"tmpc_{qb}")
                t3c = wt(f"t3c_{qb}", BF, 3)
                acol_ = lambda c: tri(s.Ap, c, stride=3 * FD)
                for i_s, (a, b) in enumerate(((0, 1), (0, 2), (1, 2))):
                    dst = ent(s.Cp, 3 + i_s)
                    tt(AL.mult, tri(t3c), acol_(a), acol_(b))
                    tt(AL.add, dst, ent(t3c, 0), ent(t3c, 1))
                    tt(AL.add, dst, dst, ent(t3c, 2))
                s.cE = lambda i_s: ent(s.Cp, i_s)
                u0, u1, u2 = wt(f"u0_{qb}"), wt(f"u1_{qb}"), wt(f"u2_{qb}")
                detA = wt(f"detA_{qb}")
                tt(AL.mult, ent(u0), aE(1, 1), aE(2, 2))
                tt(AL.mult, ent(tmpb), aE(2, 1), aE(1, 2))
                tt(AL.subtract, ent(u0), ent(u0), ent(tmpb))
                tt(AL.mult, ent(u1), aE(0, 1), aE(2, 2))
                tt(AL.mult, ent(tmpb), aE(2, 1), aE(0, 2))
                tt(AL.subtract, ent(u1), ent(u1), ent(tmpb))
                tt(AL.mult, ent(u2), aE(0, 1), aE(1, 2))
                tt(AL.mult, ent(tmpb), aE(1, 1), aE(0, 2))
                tt(AL.subtract, ent(u2), ent(u2), ent(tmpb))
                tt(AL.mult, ent(detA), aE(0, 0), ent(u0))
                tt(AL.mult, ent(tmpb), aE(1, 0), ent(u1))
                tt(AL.subtract, ent(detA), ent(detA), ent(tmpb))
                tt(AL.mult, ent(tmpb), aE(2, 0), ent(u2))
                tt(AL.add, ent(detA), ent(detA), ent(tmpb))
                s.detA = detA
                s.sqb3 = wpool.tile([P, 3 * FD], BF, tag="sqb",
                                    uniquify=True, bufs=2, name=f"sqb{qb}")
                act.square(s.sqb3[:, :], s.Cp[:, 3 * FD:6 * FD])

            def S2():
                """p1, tr, qm, b3, cross-products; issue sb3 square."""
                s.sq01, s.sq02, s.sq12 = (ent(s.sqb3, 0), ent(s.sqb3, 1),
                                          ent(s.sqb3, 2))
                s.p1 = xt(f"p1_{qb}")
                tt(AL.add, ent(s.p1), s.sq01, s.sq02)
                tt(AL.add, ent(s.p1), ent(s.p1), s.sq12)
                s.trb = xt(f"trb_{qb}")
                tt(AL.add, ent(s.trb), s.cE(0), s.cE(1))
                tt(AL.add, ent(s.trb), ent(s.trb), s.cE(2))
                s.qm = xt(f"qm_{qb}")
                act.mul(s.qm[:, :], s.trb[:, :], 1.0 / 3.0)
                s.b3 = wt(f"b3_{qb}", BF, 3)
                tt(AL.subtract, tri(s.b3), tri(s.Cp, 0), bc(ent(s.qm), 3))
                s.cp01 = xt(f"cp01_{qb}")
                s.cp02 = xt(f"cp02_{qb}")
                s.cp12 = xt(f"cp12_{qb}")
                tt(AL.mult, ent(s.cp01), s.cE(4), s.cE(5))
                tt(AL.mult, ent(s.cp02), s.cE(3), s.cE(5))
                tt(AL.mult, ent(s.cp12), s.cE(3), s.cE(4))
                s.sb3 = wt(f"sb3_{qb}", BF, 3)
                act.square(s.sb3[:, :], s.b3[:, :])

            def S3():
                """p2; issue ln/exp block; detC as filler."""
                s.p2 = wt(f"p2_{qb}")
                tt(AL.add, ent(s.p2), ent(s.sb3, 0), ent(s.sb3, 1))
                tt(AL.add, ent(s.p2), ent(s.p2), ent(s.sb3, 2))
                vec.scalar_tensor_tensor(out=ent(s.p2), in0=ent(s.p1),
                                         scalar=2.0, in1=ent(s.p2),
                                         op0=AL.mult, op1=AL.add)
                vec.tensor_scalar_max(out=s.p2[:, :], in0=s.p2[:, :],
                                      scalar1=1e-12)
                s.lnp6 = wt(f"lnp6_{qb}", F32)
                act.activation(s.lnp6[:, :], s.p2[:, :], AF.Ln,
                               scale=4.0 / 6.0)
                s.two_p = wt(f"two_p_{qb}", F32)
                act.activation(s.two_p[:, :], s.lnp6[:, :], AF.Exp,
                               scale=0.5)
                s.pinv8 = wt(f"pinv8_{qb}", F32)
                act.activation(s.pinv8[:, :], s.lnp6[:, :], AF.Exp,
                               scale=-1.5)
                s.two_pb = wt(f"two_pb_{qb}")
                act.copy(s.two_pb[:, :], s.two_p[:, :])
                # detC (DVE filler, independent of the ACT chain)
                b0, b1, b2 = ent(s.b3, 0), ent(s.b3, 1), ent(s.b3, 2)
                tmpb = wt(f"tmpd_{qb}")
                ub0, ub1, ub2 = (wt(f"ub0_{qb}"), wt(f"ub1_{qb}"),
                                 wt(f"ub2_{qb}"))
                tt(AL.mult, ent(ub0), b1, b2)
                tt(AL.subtract, ent(ub0), ent(ub0), s.sq12)
                tt(AL.mult, ent(ub1), s.cE(3), b2)
                tt(AL.subtract, ent(ub1), ent(ub1), ent(s.cp01))
                tt(AL.mult, ent(ub2), b1, s.cE(4))
                tt(AL.subtract, ent(ub2), ent(s.cp02), ent(ub2))
                s.detC = wt(f"detC_{qb}")
                tt(AL.mult, ent(s.detC), b0, ent(ub0))
                tt(AL.mult, ent(tmpb), s.cE(3), ent(ub1))
                tt(AL.subtract, ent(s.detC), ent(s.detC), ent(tmpb))
                tt(AL.mult, ent(tmpb), s.cE(4), ent(ub2))
                tt(AL.add, ent(s.detC), ent(s.detC), ent(tmpb))

            def S4():
                """r; issue r2/lnomr/eh."""
                s.r = wt(f"r_{qb}", F32)
                vec.scalar_tensor_tensor(out=ent(s.r), in0=ent(s.detC),
                                         scalar=4.0, in1=ent(s.pinv8),
                                         op0=AL.mult, op1=AL.mult)
                vec.tensor_scalar(out=s.r[:, :], in0=s.r[:, :],
                                  scalar1=RCLAMP, scalar2=-RCLAMP,
                                  op0=AL.min, op1=AL.max)
                r2 = wt(f"r2_{qb}", F32)
                act.square(r2[:, :], s.r[:, :])
                lnomr = wt(f"lnomr_{qb}", F32)
                act.activation(lnomr[:, :], r2[:, :], AF.Ln, bias=1.0,
                               scale=-1.0)
                s.eh = wt(f"eh_{qb}", F32)
                act.activation(s.eh[:, :], lnomr[:, :], AF.Exp, scale=-0.5)

            def S5s():
                s.s_ = wt(f"s__{qb}", F32)
                tt(AL.mult, ent(s.s_), ent(s.r), ent(s.eh))

            def S5at():
                s.at = wt(f"at_{qb}", F32)
                act.activation(s.at[:, :], s.s_[:, :], AF.Arctan)

            def S5sin():
                s.sinL = wt(f"sinL_{qb}")
                act.activation(s.sinL[:, :], s.at[:, :], AF.Sin,
                               bias=bias_sinl[:, :], scale=-1.0 / 3.0)
                s.sinM = wt(f"sinM_{qb}")
                act.activation(s.sinM[:, :], s.at[:, :], AF.Sin,
                               scale=-1.0 / 3.0)

            def S6():
                """eigenvalues, gaps, clamps; issue the g/recip ACT block."""
                tmpb = wt(f"tmpe_{qb}")
                s.lam3, s.lam1 = xt(f"lam3_{qb}"), xt(f"lam1_{qb}")
                lam2 = wt(f"lam2_{qb}")
                tt(AL.mult, ent(tmpb), ent(s.two_pb), ent(s.sinL))
                tt(AL.add, ent(s.lam3), ent(s.qm), ent(tmpb))
                tt(AL.mult, ent(tmpb), ent(s.two_pb), ent(s.sinM))
                tt(AL.add, ent(lam2), ent(s.qm), ent(tmpb))
                tt(AL.subtract, ent(tmpb), ent(s.trb), ent(s.lam3))
                tt(AL.subtract, ent(s.lam1), ent(tmpb), ent(lam2))
                tt(AL.subtract, ent(tmpb), ent(s.sinL), ent(s.sinM))
                # d-quad = [d21, d31, ssum(later), d32]
                s.dq = wt(f"dq_{qb}", BF, 4, tag="q4", bufs=8)
                tt(AL.subtract, ent(s.dq, 0), ent(lam2), ent(s.lam1))
                tt(AL.subtract, ent(s.dq, 1), ent(s.lam3), ent(s.lam1))
                tt(AL.mult, ent(s.dq, 3), ent(s.two_pb), ent(tmpb))
                # l-pair = [max(lam2, clip), max(lam3, clip)]
                s.lp = wt(f"lp_{qb}", BF, 2)
                vec.tensor_scalar_max(out=s.lp[:, 0:FD], in0=lam2[:, :],
                                      scalar1=CLIPV)
                vec.tensor_scalar_max(out=s.lp[:, FD:2 * FD],
                                      in0=s.lam3[:, :], scalar1=CLIPV)
                # eps-quad = [l3q, l3q, l3c, l3q] (x^2-scales of dq slots)
                s.epsq = wt(f"epsq_{qb}", BF, 4, tag="q4", bufs=8)
                vec.tensor_copy(s.epsq[:, 2 * FD:3 * FD],
                                s.lp[:, FD:2 * FD])
                # ACT block (all natural_log_exp set): g-pair, l3^4
                lnl = wt(f"lnl_{qb}", BF, 2)
                act.activation(lnl[:, :], s.lp[:, :], AF.Ln)
                s.gP = wt(f"gP_{qb}", BF, 2)
                act.activation(s.gP[:, :], lnl[:, :], AF.Exp, scale=-0.5)
                l3sq = wt(f"l3sq_{qb}")
                act.square(l3sq[:, :], s.lp[:, FD:2 * FD])
                s.l3q = wt(f"l3q_{qb}")
                act.square(s.l3q[:, :], l3sq[:, :])

            def S7a():
                """[needs gP, l3q] ssum/q23/eps-quad; issue square(dq)."""
                s.sq3 = wt(f"sq3_{qb}")
                tt(AL.mult, ent(s.sq3), ent(s.lp, 1), ent(s.gP, 1))
                tmps = wt(f"tmps_{qb}")
                tt(AL.mult, ent(tmps), ent(s.lp, 0), ent(s.gP, 0))
                tt(AL.add, ent(s.dq, 2), ent(tmps), ent(s.sq3))
                s.q23 = wt(f"q23_{qb}")
                tt(AL.mult, ent(s.q23), ent(s.gP, 0), ent(s.gP, 1))
                s.dsg = xt(f"dsg_{qb}")
                act.sign(s.dsg[:, :], s.detA[:, :])
                s.selb = xt(f"selb_{qb}")
                act.activation(s.selb[:, :], s.dsg[:, :], AF.Copy, bias=0.5,
                               scale=0.5)
                lq = s.l3q[:, :]
                lqb = bass.AP(lq.tensor, lq.offset,
                              [list(lq.ap[0]), [0, 2], [1, FD]])
                dst01 = s.epsq[:, 0:2 * FD]
                vec.tensor_copy(bass.AP(dst01.tensor, dst01.offset,
                                        [list(dst01.ap[0]), [FD, 2],
                                         [1, FD]]), lqb)
                vec.tensor_copy(s.epsq[:, 3 * FD:4 * FD], lq)
                s.sqq = wt(f"sqq_{qb}", BF, 4, tag="q4", bufs=8)
                act.square(s.sqq[:, :], s.dq[:, :])

            def S7a2():
                """[needs sqq] eps-add + floor; issue ln/exp quad."""
                vec.scalar_tensor_tensor(out=s.sqq[:, :],
                                         in0=s.epsq[:, :],
                                         scalar=1e-12, in1=s.sqq[:, :],
                                         op0=AL.mult, op1=AL.add)
                vec.tensor_scalar_max(out=s.sqq[:, :], in0=s.sqq[:, :],
                                      scalar1=1e-30)
                act.activation(s.sqq[:, :], s.sqq[:, :], AF.Ln)
                s.eiq = wt(f"eiq_{qb}", BF, 4, tag="q4", bufs=8)
                act.activation(s.eiq[:, :], s.sqq[:, :], AF.Exp,
                               scale=-1.0)

            def S7b():
                """gam's, T2, W2, Y, R, energy residuals (big DVE block)."""
                # N1/Md/T2 first: независимы of the reciprocal quad, so the
                # DVE has work while ACT finishes the ln/exp for iq.
                N1p = wt(f"N1p_{qb}", BF, 6)
                tt(AL.subtract, tri(N1p, 0), tri(s.Cp, 0),
                   bc(ent(s.lam1), 3))
                vec.tensor_copy(N1p[:, 3 * FD:6 * FD],
                                s.Cp[:, 3 * FD:6 * FD])
                Md3 = wt(f"Md3_{qb}", BF, 3)
                tt(AL.subtract, tri(Md3), tri(s.Cp, 0), bc(ent(s.lam3), 3))
                T2p = wt(f"T2p_{qb}", BF, 6)
                tt(AL.mult, tri(T2p, 0), tri(N1p, 0), tri(Md3))
                tt(AL.add, ent(T2p, 0), ent(T2p, 0), s.sq01)
                tt(AL.add, ent(T2p, 0), ent(T2p, 0), s.sq02)
                tt(AL.add, ent(T2p, 1), ent(T2p, 1), s.sq01)
                tt(AL.add, ent(T2p, 1), ent(T2p, 1), s.sq12)
                tt(AL.add, ent(T2p, 2), ent(T2p, 2), s.sq02)
                tt(AL.add, ent(T2p, 2), ent(T2p, 2), s.sq12)
                tq = wt(f"tq_{qb}")
                for (slot, a, mslot, cslot, cpx) in (
                        (3, 0, 1, 3, s.cp01), (4, 0, 2, 4, s.cp02),
                        (5, 1, 2, 5, s.cp12)):
                    tt(AL.add, ent(tq), ent(N1p, a), ent(Md3, mslot))
                    tt(AL.mult, ent(T2p, slot), s.cE(cslot), ent(tq))
                    tt(AL.add, ent(T2p, slot), ent(T2p, slot), ent(cpx))
                # i-quad = [1/d21, 1/d31, 1/ssum, 1/d32]
                iq = s.eiq
                vec.tensor_tensor(out=iq[:, :], in0=s.dq[:, :],
                                  in1=iq[:, :], op=AL.mult)
                s.p2131 = wt(f"p2131_{qb}")
                tt(AL.mult, ent(s.p2131), ent(iq, 0), ent(iq, 1))
                c3, c4 = wt(f"c3_{qb}"), wt(f"c4_{qb}")
                t1 = wt(f"t1_{qb}")
                # c3+ = -(d21/ssum + sq3) * q23 * i21*i31
                tt(AL.mult, ent(t1), ent(s.dq, 0), ent(iq, 2))
                tt(AL.add, ent(t1), ent(t1), ent(s.sq3))
                tt(AL.mult, ent(t1), ent(t1), ent(s.q23))
                c3p = wt(f"c3p_{qb}")
                vec.scalar_tensor_tensor(out=ent(c3p), in0=ent(t1),
                                         scalar=-1.0, in1=ent(s.p2131),
                                         op0=AL.mult, op1=AL.mult)
                # c3- = -(g3*d21 + g2*d31) / (d21*d31*d32)
                t2 = wt(f"t2_{qb}")
                tt(AL.mult, ent(t2), ent(s.gP, 1), ent(s.dq, 0))
                tt(AL.mult, ent(t1), ent(s.gP, 0), ent(s.dq, 1))
                tt(AL.add, ent(t2), ent(t2), ent(t1))
                tt(AL.mult, ent(t2), ent(t2), ent(iq, 3))
                c3m = wt(f"c3m_{qb}")
                vec.scalar_tensor_tensor(out=ent(c3m), in0=ent(t2),
                                         scalar=-1.0, in1=ent(s.p2131),
                                         op0=AL.mult, op1=AL.mult)
                # blend on sign: c3 = c3m + (dsg+1)/2 * (c3p - c3m)
                selb = s.selb
                tt(AL.subtract, ent(t1), ent(c3p), ent(c3m))
                tt(AL.mult, ent(t1), ent(selb), ent(t1))
                tt(AL.add, ent(c3), ent(c3m), ent(t1))
                # c4 = dsg * g3 * i31
                tt(AL.mult, ent(c4), ent(s.gP, 1), ent(iq, 1))
                tt(AL.mult, ent(c4), ent(s.dsg), ent(c4))
                # W2 = c3*T2 + c4*N1 (in place on T2p)
                t6 = wt(f"t6_{qb}", BF, 6)
                tt(AL.mult, six(t6), six(N1p), bc(ent(c4), 6))
                tt(AL.mult, six(T2p), six(T2p), bc(ent(c3), 6))
                tt(AL.add, six(T2p), six(T2p), six(t6))
                w2 = lambda cc, j: bc(ent(T2p, SYM[(cc, j)]), 3)
                s.dbg_W2, s.dbg_c3, s.dbg_c4 = T2p, c3, c4
                # Y = A @ W2
                Yp = wt(f"Yp_{qb}", BF, 9)
                t3 = wt(f"t3b_{qb}", BF, 3)
                acol = lambda cc: tri(s.Ap, cc, stride=3 * FD)
                ycol = lambda j: tri(Yp, j, stride=3 * FD)
                for j in range(3):
                    tt(AL.mult, ycol(j), acol(0), w2(0, j))
                    for cc in (1, 2):
                        tt(AL.mult, tri(t3), acol(cc), w2(cc, j))
                        tt(AL.add, ycol(j), ycol(j), tri(t3))
                # R = Y + cof(Y)
                s.dbg_Yp = Yp
                Rp = wt(f"Rp_{qb}", BF, 9)
                s.dbg_Rp = Rp
                yE = lambda i, j: ent(Yp, i * 3 + j)
                cf = wt(f"cf_{qb}")
                tmpb = wt(f"tmpf_{qb}")
                for i in range(3):
                    for j in range(3):
                        i1, i2 = (i + 1) % 3, (i + 2) % 3
                        j1, j2 = (j + 1) % 3, (j + 2) % 3
                        tt(AL.mult, ent(cf), yE(i1, j1), yE(i2, j2))
                        tt(AL.mult, ent(tmpb), yE(i1, j2), yE(i2, j1))
                        tt(AL.subtract, ent(cf), ent(cf), ent(tmpb))
                        tt(AL.add, ent(Rp, i * 3 + j), yE(i, j), ent(cf))
                # energy residuals (software-pipelined with ACT squares)
                rcol = lambda j: tri(Rp, j, stride=3 * FD)
                Rpm3 = wt(f"Rpm3_{qb}", BF, 3)
                tt(AL.add, tri(Rpm3), rcol(0), rcol(1))
                Z3 = wt(f"Z3_{qb}", BF, 3)
                dfc = [wt(f"dfc{i}_{qb}", BF, 3) for i in (0, 1)]
                sqd = [wt(f"sqd{i}_{qb}", BF, 3) for i in (0, 1)]
                s.ns6 = wt(f"ns6_{qb}", BF, 6)
                combos = ((AL.add, tri(Rpm3)), (AL.add, rcol(0)),
                          (AL.add, rcol(1)), (AL.subtract, rcol(1)),
                          (AL.subtract, rcol(0)), (AL.subtract, tri(Rpm3)))

                def emit_dfc(k):
                    d = dfc[k % 2]
                    tt(AL.mult, tri(Z3), rcol(2), r_tz(k))
                    tt(AL.subtract, tri(d), s.eT(k), tri(Z3))
                    op, cv = combos[k]
                    tt(op, tri(d), tri(d), cv)
                    act.square(sqd[k % 2][:, :], d[:, :])

                def emit_ns(k):
                    sq = sqd[k % 2]
                    tt(AL.add, ent(s.ns6, k), ent(sq, 0), ent(sq, 1))
                    tt(AL.add, ent(s.ns6, k), ent(s.ns6, k), ent(sq, 2))

                emit_dfc(0)
                for k in range(1, K):
                    emit_dfc(k)
                    emit_ns(k - 1)
                emit_ns(K - 1)

            def S8sqrt():
                if dbg_n and qb == 0:
                    FDl = FD

                    def dump(slot, t, n):
                        nc.sync.dma_start(
                            dbg_d[:, slot * FDl:(slot + n) * FDl],
                            t[:, 0:n * FDl])
                    dump(0, s.Ap, 9)
                    dump(9, s.Cp, 6)
                    dump(15, s.gP, 2)
                    dump(17, s.dbg_c3, 1)
                    dump(18, s.dbg_c4, 1)
                    dump(19, s.lam1, 1)
                    dump(20, s.lam3, 1)
                    dump(21, s.dbg_W2, 6)
                    dump(27, s.dbg_Yp, 9)
                    dump(36, s.dbg_Rp, 9)
                    dump(45, s.Et, 3)
                    dump(48, s.dsg, 1)
                    dump(49, s.ns6, 6)
                    dump(55, s.dp, 2)
                    dump(57, s.dsq, 2)
                    dump(59, s.ei, 2)
                    dump(61, s.l3q, 1)
                act.activation(s.ns6[:, :], s.ns6[:, :], AF.Sqrt)

            def S8():
                tt(AL.mult, six(s.ns6), six(s.ns6), wrow6())
                s3t = wt(f"s3_{qb}", BF, 3)
                tt(AL.add, tri(s3t), tri(s.ns6, 0), tri(s.ns6, 3))
                nrg = wt(f"nrg_{qb}")
                tt(AL.add, ent(nrg), ent(s3t, 0), ent(s3t, 1))
                tt(AL.add, ent(nrg), ent(nrg), ent(s3t, 2))
                vec.tensor_scalar_min(out=nrg[:, :], in0=nrg[:, :],
                                      scalar1=1.0)
                vec.tensor_reduce(out=outacc[:, qb * BQ:(qb + 1) * BQ],
                                  in_=ent(nrg), axis=mybir.AxisListType.X,
                                  op=AL.add)

            return [S0a, S1, S2, S3, S4, S5s, S5at, S5sin, S6, S7a,
                    S7a2, S7b, S8sqrt, S8]

        for q0 in range(0, NQ, 2):
            segsA = build_pass(q0)
            segsB = build_pass(q0 + 1)
            for sa, sb in zip(segsA, segsB):
                sa()
                sb()

        nc.sync.dma_start(out_d[:, :], outacc[:, :])

    nc.compile()
    return nc


def _get_nc(wingeo):
    if wingeo not in _nc_cache:
        _pin_act_tables()
        _nc_cache[wingeo] = _build_nc(wingeo)
    return _nc_cache[wingeo]


# ---------------------------------------------------------------------------
# Entry point
# ---------------------------------------------------------------------------

def _install_ntff_shim():
    """Provide antenv.axon_hooks (missing in this image) so
    run_bass_kernel_spmd(trace=True) can reach the NTFF profiler in
    libaxon_pjrt.so."""
    import types

    try:
        import antenv.axon_hooks  # noqa: F401
        return True
    except ImportError:
        pass
    try:
        import antenv
        from trn_agent_boot.trn_boot import _ntff_profile_via_ctypes
    except ImportError:
        return False
    mod = types.ModuleType("antenv.axon_hooks")
    state = {"hook": None}
    mod.set_axon_ntff_profile_hook = lambda h: state.__setitem__("hook", h)
    mod.get_axon_ntff_profile_hook = lambda: state["hook"]
    sys.modules["antenv.axon_hooks"] = mod
    antenv.axon_hooks = mod
    try:
        hook = _ntff_profile_via_ctypes("/opt/axon/libaxon_pjrt.so")
    except OSError:
        hook = None
    if hook is not None:
        mod.set_axon_ntff_profile_hook(hook)
    return hook is not None


def kernel(**inputs) -> np.ndarray:
    pred = np.asarray(inputs["prediction"], np.float32)
    adj_idx = np.asarray(inputs["adj_list_indices"])
    adj_w = np.asarray(inputs["adj_list_weights"], np.float32)
    tev_T = np.asarray(inputs["template_edge_vectors_T"], np.float32)

    wk, tzk = _build_offset_classes(adj_idx, adj_w, tev_T)
    in_maps, wingeo = _host_prepare(pred, wk, tzk)

    nc = _get_nc(wingeo)
    import os
    trace = bool(int(os.environ.get("ARAP_TRACE", "0")))
    if trace:
        trace = _install_ntff_shim()
    try:
        res = run_bass_kernel_spmd(nc, in_maps, core_ids=list(range(NCORES)),
                                   trace=trace)
    except Exception:
        if not trace:
            raise
        res = run_bass_kernel_spmd(nc, in_maps, core_ids=list(range(NCORES)),
                                   trace=False)
    kernel._last_exec_ns = res.exec_time_ns
    kernel._last_results = res

    total = np.zeros(B, np.float64)
    for c in range(NCORES):
        total += res.results[c]["out"].astype(np.float64).sum(axis=0)
    return (total / NV).astype(np.float32)


kernel._last_exec_ns = None


# revision 31
# speedup vs baseline: 1.0248x; 1.0242x over previous
"""ARAP loss kernel for Trainium2 (8 NeuronCores, SPMD over the vertex axis).

Problem: nn_ArapLoss — per-vertex 6-neighbor gather on a 316x316 grid mesh,
3x3 polar decomposition (closed-form symmetric eigenanalysis) per vertex,
cotan-weighted edge-residual energy, clamped mean over vertices.

Strategy (v3 — vector-engine lean, pair-pipelined)
--------------------------------------------------
- Shard the vertex axis N=99856 across 8 cores (12482 each, padded to
  12544 = 128*98).  Grid adjacency reduces to K=6 constant index offsets
  {+-1, +-316, +-317}; the host materializes shifted windows of
  `prediction` so the device does no gather.
- Edge vectors e_k = q_{n+o_k} - p_n are computed ONCE in f32 and stored
  bf16; everything downstream runs in bf16 (DVE 2x mode).
- The template-edge xy components are EXACTLY {0,+-1} per offset class
  (regular grid), so A = sum_k e_k (stab w_k t_k)^T collapses to signed
  sums plus one weighted z-column, and the rotated-template residual
  e_k - R t_k collapses to (e_k - tz_k R[:,2]) -+ R-column combos.
- R from a SINGLE 3x3 product:  R = Y + cof(Y),  Y = A (g2 P2 + d g3 P3).
  cof(u2 v2' + d u3 v3') = d^2 u1 v1' = u1 v1', so the smallest-eigenvalue
  component needs no division by s1 and no second product / sign fixup.
- Safe reciprocals as x/(x^2 + eps) — no Abs/Sign, sign rides the x.
- Passes are emitted in PAIRS, interleaved at every ACT-dependency
  boundary, so the in-order DVE queue always has independent work while
  the scalar engine walks its serial ln/exp/arctan/sin chain.  The trig
  and sqrt activations of the two passes share one table-load block.
- Output: per-core partial sums [128, B]; host reduces and divides by N.
"""
import sys

for _p in ("/opt/trn_rl_repo", "/opt/trn_rl_repo/concourse", "/opt/pypackages"):
    if _p not in sys.path:
        sys.path.insert(0, _p)

from types import SimpleNamespace

from contextlib import ExitStack

import ml_dtypes
import numpy as np

import concourse.bass as bass
import concourse.tile as tile
from concourse import bacc, mybir
from concourse.bass_utils import run_bass_kernel_spmd

F32 = mybir.dt.float32
BF = mybir.dt.bfloat16
AL = mybir.AluOpType
AF = mybir.ActivationFunctionType

# ---- problem geometry (hardcoded per spec) --------------------------------
B = 16
NV = 99856
NCORES = 8
P = 128
NC_V = NV // NCORES            # 12482 real vertices per core
FQ = 98                        # free-dim vertices per partition
VP = P * FQ                    # 12544 padded vertices per core
BQ = 4                         # batch elements per pass
NQ = B // BQ
K = 6
CLIPV = 1e-12                  # eigenvalue clamp (unscaled units)
C_SINL = float(2.0 * np.pi / 3.0)
RCLAMP = 1.0 - 1e-6
OFFS = (-317, -316, -1, 1, 316, 317)
# xy components of template edges per offset class (exact on the grid)
CX = (-1, -1, 0, 0, 1, 1)
CY = (-1, 0, -1, 1, 0, 1)

_nc_cache = {}


def _pin_act_tables():
    """Shrink the cached activation-table membership map so the compiler
    assigns Ln/Exp/Square/Sign/Abs/Copy to the one combined set that
    physically contains them all (natural_log_exp_and_others), Sin to
    trig_and_small, Arctan to sigmoid_and_others.  Every set we leave a
    function in really does contain it, so the emitted table loads stay
    valid — this only stops the compiler from ping-ponging between the
    ln-only and exp-only sets.  Best effort: on any surprise, leave the
    tables untouched (costs extra table loads, still correct)."""
    try:
        from concourse.hw_specs import get_activation_tables
        tabs = None
        for arch in ("gen3", "TRN2"):
            try:
                tabs = get_activation_tables(arch)
                break
            except Exception:
                continue
        if tabs is None:
            return
        combined = tabs.get("natural_log_exp_and_others")
        shared = {AF.Ln, AF.Exp, AF.Square, AF.Sign, AF.Abs, AF.Copy,
                  AF.Identity}
        if combined is None or not (shared <= combined):
            return
        if AF.Sin not in tabs.get("trig_and_small", set()):
            return
        for name, fns in tabs.items():
            if name == "natural_log_exp_and_others":
                continue
            fns -= shared
            if name != "trig_and_small":
                fns.discard(AF.Sin)
    except Exception:
        pass


# ---------------------------------------------------------------------------
# Host-side preprocessing
# ---------------------------------------------------------------------------

def _build_offset_classes(adj_idx, adj_w, tev_T):
    """(N,D) adjacency -> per-offset-class weights wk (K,N) and template
    edge z-components tzk (K,N).  Asserts the grid structure this kernel
    hardcodes (xy components == CX/CY per class)."""
    N, D = adj_idx.shape
    ar = np.arange(N, dtype=np.int64)
    real = (adj_idx > 0) | (np.arange(D)[None, :] == 0)
    delta = np.asarray(adj_idx, np.int64) - ar[:, None]
    offs = np.unique(delta[real])
    assert tuple(int(o) for o in offs) == OFFS, f"unexpected offsets {offs}"
    wk = np.zeros((K, N), np.float32)
    tzk = np.zeros((K, N), np.float32)
    for k, o in enumerate(OFFS):
        sel = real & (delta == o)
        n_id, d_id = np.nonzero(sel)
        wk[k, n_id] = adj_w[n_id, d_id]
        tzk[k, n_id] = tev_T[n_id, 2, d_id]
    return wk, tzk


def _group_offsets(gap=8):
    """Group [0]+OFFS into consecutive runs; returns (bases, width, win_map)
    where win_map[x] = (g, slot) for x in [0(center)] + OFFS order."""
    allo = sorted(set([0] + list(OFFS)))
    groups = [[allo[0]]]
    for o in allo[1:]:
        if o - groups[-1][-1] <= gap:
            groups[-1].append(o)
        else:
            groups.append([o])
    bases = [g[0] for g in groups]
    width = FQ + max(g[-1] - g[0] for g in groups) + 1
    lut = {}
    for gi, g in enumerate(groups):
        for o in g:
            lut[o] = (gi, o - g[0])
    win_map = [lut[0]] + [lut[o] for o in OFFS]
    return bases, width, tuple(win_map)


def _host_prepare(pred, wk, tzk):
    """Build per-core input maps: predl [P, G*B*3*GWD] f32 (group-major so
    each pass loads G contiguous chunks) and constb [P, 24*FQ] bf16
    (rows: wp(6), wz(6), tz(6), wk(6))."""
    bases, GWD, win_map = _group_offsets()
    G = len(bases)
    H = max(max(abs(o) for o in OFFS), 1)
    padlen = NV + 2 * H + (VP - NC_V) + GWD
    padG = np.zeros((B, 3, padlen), np.float32)
    padG[:, :, H:H + NV] = pred

    wp = wk                              # (K, N) — no stab scaling: R is
    wz = wp * tzk                        # scale-invariant; bf16 ranges stay sane
    CG = np.concatenate([wp, wz, tzk, wk], axis=0)   # (24, N)

    in_maps = []
    pidx = (np.arange(P)[:, None] * FQ + np.arange(GWD)[None, :])  # (P,GWD)
    for c in range(NCORES):
        base = c * NC_V
        wins = np.empty((G, B, 3, P, GWD), np.float32)
        for g, bg in enumerate(bases):
            idx = H + base + bg + pidx
            wins[g] = padG[:, :, idx].transpose(0, 1, 2, 3)
        predl = np.ascontiguousarray(
            wins.transpose(3, 0, 1, 2, 4)
        ).reshape(P, G * B * 3 * GWD)

        cc = np.zeros((24, VP), np.float32)
        hi = min(base + VP, NV) - base
        hi = min(hi, NC_V)                   # zero weights on padded tail
        cc[:, :hi] = CG[:, base:base + hi]
        constb = np.ascontiguousarray(
            cc.reshape(24, P, FQ).transpose(1, 0, 2)
        ).reshape(P, 24 * FQ).astype(ml_dtypes.bfloat16)

        in_maps.append({"predl": predl, "constb": constb})
    return in_maps, (G, GWD, win_map)


# ---------------------------------------------------------------------------
# Device kernel builder
# ---------------------------------------------------------------------------

def _build_nc(wingeo):
    G, GWD, win_map = wingeo
    FD = BQ * FQ

    nc = bacc.Bacc("TRN2", target_bir_lowering=False, debug=False,
                   num_devices=NCORES)

    predl_d = nc.dram_tensor("predl", [P, G * B * 3 * GWD], F32,
                             kind="ExternalInput").ap()
    constb_d = nc.dram_tensor("constb", [P, 24 * FQ], BF,
                              kind="ExternalInput").ap()
    out_d = nc.dram_tensor("out", [P, B], F32, kind="ExternalOutput").ap()
    import os
    dbg_n = int(os.environ.get("ARAP_DBG", "0"))
    dbg_d = (nc.dram_tensor("dbg", [P, dbg_n * BQ * FQ], BF,
                            kind="ExternalOutput").ap() if dbg_n else None)

    with tile.TileContext(nc) as tc, ExitStack() as ctx:
        cpool = ctx.enter_context(tc.tile_pool(name="consts", bufs=1))
        ppool = ctx.enter_context(tc.tile_pool(name="pred", bufs=2))
        wpool = ctx.enter_context(tc.tile_pool(name="work", bufs=96))

        cb = cpool.tile([P, 24 * FQ], BF)
        nc.scalar.dma_start(cb[:, :], constb_d[:, :])
        outacc = cpool.tile([P, B], F32)
        bias_sinl = cpool.tile([P, 1], F32)
        nc.gpsimd.memset(bias_sinl[:, :], C_SINL)

        vec = nc.vector
        act = nc.scalar

        def crow3(r):
            """bf16 const row r as [P, 3, BQ, FQ] (i- and batch-bcast)."""
            a = cb[:, r * FQ:(r + 1) * FQ]
            return bass.AP(a.tensor, a.offset,
                           [list(a.ap[0]), [0, 3], [0, BQ], list(a.ap[1])])

        def wrow6():
            """wk rows 18..23 as [P, 6, BQ, FQ]."""
            a = cb[:, 18 * FQ:24 * FQ]
            return bass.AP(a.tensor, a.offset,
                           [list(a.ap[0]), [FQ, 6], [0, BQ], [1, FQ]])

        r_wp = lambda k: crow3(k)
        r_wz = lambda k: crow3(6 + k)
        r_tz = lambda k: crow3(12 + k)

        def tt(op, out, a, b):
            vec.tensor_tensor(out=out, in0=a, in1=b, op=op)

        def wt(name, dt=BF, n=1, tag=None, bufs=None):
            if tag is None:
                tag = {(BF, 1): "sg", (F32, 1): "sf", (BF, 2): "p2k",
                       (BF, 3): "t3", (BF, 6): "s6", (BF, 9): "pk9"}[
                           (dt, n)]
            if bufs is None:
                bufs = {"sg": 19, "sf": 8, "p2k": 8, "t3": 7, "s6": 4,
                        "pk9": 3, "x2": 18, "q4": 8}[tag]
            return wpool.tile([P, n * FD], dt, tag=tag, name=name,
                              uniquify=True, bufs=bufs)

        def xt(name):
            """long-lived per-pass single (2 passes in flight)."""
            return wt(name, BF, 1, tag="x2", bufs=18)

        def ent(t, s=0):
            a = t[:, :]
            return bass.AP(a.tensor, a.offset + s * FD,
                           [list(a.ap[0]), [FQ, BQ], [1, FQ]])

        def tri(t, s=0, stride=FD):
            a = t[:, :]
            return bass.AP(a.tensor, a.offset + s * FD,
                           [list(a.ap[0]), [stride, 3], [FQ, BQ], [1, FQ]])

        def pair(t):
            a = t[:, :]
            return bass.AP(a.tensor, a.offset,
                           [list(a.ap[0]), [FD, 2], [FQ, BQ], [1, FQ]])

        def six(t):
            a = t[:, :]
            return bass.AP(a.tensor, a.offset,
                           [list(a.ap[0]), [FD, 6], [FQ, BQ], [1, FQ]])

        def bc(x, n):
            """broadcast a [P, BQ, FQ] view over n."""
            return bass.AP(x.tensor, x.offset,
                           [list(x.ap[0]), [0, n]] +
                           [list(d) for d in x.ap[1:]])

        SYM = {(0, 0): 0, (1, 1): 1, (2, 2): 2,
               (0, 1): 3, (1, 0): 3, (0, 2): 4, (2, 0): 4,
               (1, 2): 5, (2, 1): 5}

        def build_pass(qb):
            s = SimpleNamespace(qb=qb)

            def S0a():
                """DMA windows, e_k, A build; issue sqA square."""
                s.pq = [ppool.tile([P, BQ * 3 * GWD], F32, tag=f"pq{g}",
                                   uniquify=True, bufs=1,
                                   name=f"pq{g}_{qb}")
                        for g in range(G)]
                span = BQ * 3 * GWD
                for g in (1, 0, 2):
                    off = (g * B + qb * BQ) * 3 * GWD
                    nc.sync.dma_start(s.pq[g][:, :],
                                      predl_d[:, off:off + span])

                def qv3(w):
                    g, slot = win_map[w]
                    a = s.pq[g][:, :]
                    return bass.AP(a.tensor, a.offset + slot,
                                   [list(a.ap[0]), [GWD, 3],
                                    [3 * GWD, BQ], [1, FQ]])

                s.Et = wpool.tile([P, 18 * FD], BF, tag="E", uniquify=True,
                                  bufs=2, name=f"E{qb}")
                s.eT = lambda k: tri(s.Et, 3 * k)
                for k in (2, 3, 0, 1, 4, 5):
                    tt(AL.subtract, s.eT(k), qv3(k + 1), qv3(0))

                H0 = wt(f"H0_{qb}", BF, 3)
                H5 = wt(f"H5_{qb}", BF, 3)
                tt(AL.mult, tri(H0), s.eT(0), r_wp(0))
                tt(AL.mult, tri(H5), s.eT(5), r_wp(5))
                s.Ap = wpool.tile([P, 9 * FD], BF, tag="A", uniquify=True,
                                  bufs=2, name=f"A{qb}")
                t3 = wt(f"t3a_{qb}", BF, 3)
                col = lambda j: tri(s.Ap, j, stride=3 * FD)
                tt(AL.mult, col(0), s.eT(4), r_wp(4))
                tt(AL.add, col(0), col(0), tri(H5))
                tt(AL.subtract, col(0), col(0), tri(H0))
                tt(AL.mult, tri(t3), s.eT(1), r_wp(1))
                tt(AL.subtract, col(0), col(0), tri(t3))
                tt(AL.mult, col(1), s.eT(3), r_wp(3))
                tt(AL.add, col(1), col(1), tri(H5))
                tt(AL.subtract, col(1), col(1), tri(H0))
                tt(AL.mult, tri(t3), s.eT(2), r_wp(2))
                tt(AL.subtract, col(1), col(1), tri(t3))
                tt(AL.mult, col(2), s.eT(0), r_wz(0))
                for k in range(1, K):
                    tt(AL.mult, tri(t3), s.eT(k), r_wz(k))
                    tt(AL.add, col(2), col(2), tri(t3))
                s.aE = lambda i, j: ent(s.Ap, i * 3 + j)
                s.sqA = wt(f"sqA_{qb}", BF, 9)
                act.square(s.sqA[:, :], s.Ap[:, :])

            def S1():
                """C = A^T A, detA; issue dsg sign + offdiag squares."""
                aE = s.aE
                s.Cp = wpool.tile([P, 6 * FD], BF, tag="C", uniquify=True,
                                  bufs=2, name=f"C{qb}")
                tt(AL.add, tri(s.Cp, 0), tri(s.sqA, 0), tri(s.sqA, 3))
                tt(AL.add, tri(s.Cp, 0), tri(s.Cp, 0), tri(s.sqA, 6))
                tmpb = wt(f"tmpc_{qb}")
                t3c = wt(f"t3c_{qb}", BF, 3)
                acol_ = lambda c: tri(s.Ap, c, stride=3 * FD)
                for i_s, (a, b) in enumerate(((0, 1), (0, 2), (1, 2))):
                    dst = ent(s.Cp, 3 + i_s)
                    tt(AL.mult, tri(t3c), acol_(a), acol_(b))
                    tt(AL.add, dst, ent(t3c, 0), ent(t3c, 1))
                    tt(AL.add, dst, dst, ent(t3c, 2))
                s.cE = lambda i_s: ent(s.Cp, i_s)
                s.sqb3 = wpool.tile([P, 3 * FD], BF, tag="sqb",
                                    uniquify=True, bufs=2, name=f"sqb{qb}")
                act.square(s.sqb3[:, :], s.Cp[:, 3 * FD:6 * FD])

            def S2():
                """p1, tr, qm, b3, cross-products; issue sb3 square."""
                s.sq01, s.sq02, s.sq12 = (ent(s.sqb3, 0), ent(s.sqb3, 1),
                                          ent(s.sqb3, 2))
                s.p1 = xt(f"p1_{qb}")
                tt(AL.add, ent(s.p1), s.sq01, s.sq02)
                tt(AL.add, ent(s.p1), ent(s.p1), s.sq12)
                s.trb = xt(f"trb_{qb}")
                tt(AL.add, ent(s.trb), s.cE(0), s.cE(1))
                tt(AL.add, ent(s.trb), ent(s.trb), s.cE(2))
                s.qm = xt(f"qm_{qb}")
                act.mul(s.qm[:, :], s.trb[:, :], 1.0 / 3.0)
                s.b3 = wt(f"b3_{qb}", BF, 3)
                tt(AL.subtract, tri(s.b3), tri(s.Cp, 0), bc(ent(s.qm), 3))
                s.cp01 = xt(f"cp01_{qb}")
                s.cp02 = xt(f"cp02_{qb}")
                s.cp12 = xt(f"cp12_{qb}")
                tt(AL.mult, ent(s.cp01), s.cE(4), s.cE(5))
                tt(AL.mult, ent(s.cp02), s.cE(3), s.cE(5))
                tt(AL.mult, ent(s.cp12), s.cE(3), s.cE(4))
                s.sb3 = wt(f"sb3_{qb}", BF, 3)
                act.square(s.sb3[:, :], s.b3[:, :])

            def S3():
                """p2; issue ln/exp block; detC as filler."""
                s.p2 = wt(f"p2_{qb}")
                tt(AL.add, ent(s.p2), ent(s.sb3, 0), ent(s.sb3, 1))
                tt(AL.add, ent(s.p2), ent(s.p2), ent(s.sb3, 2))
                vec.scalar_tensor_tensor(out=ent(s.p2), in0=ent(s.p1),
                                         scalar=2.0, in1=ent(s.p2),
                                         op0=AL.mult, op1=AL.add)
                vec.tensor_scalar_max(out=s.p2[:, :], in0=s.p2[:, :],
                                      scalar1=1e-12)
                s.lnp6 = wt(f"lnp6_{qb}", F32)
                act.activation(s.lnp6[:, :], s.p2[:, :], AF.Ln,
                               scale=4.0 / 6.0)
                s.two_p = wt(f"two_p_{qb}", F32)
                act.activation(s.two_p[:, :], s.lnp6[:, :], AF.Exp,
                               scale=0.5)
                s.pinv8 = wt(f"pinv8_{qb}", F32)
                act.activation(s.pinv8[:, :], s.lnp6[:, :], AF.Exp,
                               scale=-1.5)
                s.two_pb = wt(f"two_pb_{qb}")
                act.copy(s.two_pb[:, :], s.two_p[:, :])
                # detC (DVE filler, independent of the ACT chain)
                b0, b1, b2 = ent(s.b3, 0), ent(s.b3, 1), ent(s.b3, 2)
                tmpb = wt(f"tmpd_{qb}")
                ub0, ub1, ub2 = (wt(f"ub0_{qb}"), wt(f"ub1_{qb}"),
                                 wt(f"ub2_{qb}"))
                tt(AL.mult, ent(ub0), b1, b2)
                tt(AL.subtract, ent(ub0), ent(ub0), s.sq12)
                tt(AL.mult, ent(ub1), s.cE(3), b2)
                tt(AL.subtract, ent(ub1), ent(ub1), ent(s.cp01))
                tt(AL.mult, ent(ub2), b1, s.cE(4))
                tt(AL.subtract, ent(ub2), ent(s.cp02), ent(ub2))
                s.detC = wt(f"detC_{qb}")
                tt(AL.mult, ent(s.detC), b0, ent(ub0))
                tt(AL.mult, ent(tmpb), s.cE(3), ent(ub1))
                tt(AL.subtract, ent(s.detC), ent(s.detC), ent(tmpb))
                tt(AL.mult, ent(tmpb), s.cE(4), ent(ub2))
                tt(AL.add, ent(s.detC), ent(s.detC), ent(tmpb))

            def S4():
                """r; issue r2/lnomr/eh."""
                s.r = wt(f"r_{qb}", F32)
                vec.scalar_tensor_tensor(out=ent(s.r), in0=ent(s.detC),
                                         scalar=4.0, in1=ent(s.pinv8),
                                         op0=AL.mult, op1=AL.mult)
                vec.tensor_scalar(out=s.r[:, :], in0=s.r[:, :],
                                  scalar1=RCLAMP, scalar2=-RCLAMP,
                                  op0=AL.min, op1=AL.max)
                r2 = wt(f"r2_{qb}", F32)
                act.square(r2[:, :], s.r[:, :])
                lnomr = wt(f"lnomr_{qb}", F32)
                act.activation(lnomr[:, :], r2[:, :], AF.Ln, bias=1.0,
                               scale=-1.0)
                s.eh = wt(f"eh_{qb}", F32)
                act.activation(s.eh[:, :], lnomr[:, :], AF.Exp, scale=-0.5)

            def S5s():
                s.s_ = wt(f"s__{qb}", F32)
                tt(AL.mult, ent(s.s_), ent(s.r), ent(s.eh))

            def S5at():
                s.at = wt(f"at_{qb}", F32)
                act.activation(s.at[:, :], s.s_[:, :], AF.Arctan)
                # detA chain here: trig-independent DVE work that fills the
                # arctan/sin table-load window (consumer dsg is in S7a)
                aE = s.aE
                u0, u1, u2 = wt(f"u0_{qb}"), wt(f"u1_{qb}"), wt(f"u2_{qb}")
                tmpa = wt(f"tmpa_{qb}")
                detA = wt(f"detA_{qb}")
                tt(AL.mult, ent(u0), aE(1, 1), aE(2, 2))
                tt(AL.mult, ent(tmpa), aE(2, 1), aE(1, 2))
                tt(AL.subtract, ent(u0), ent(u0), ent(tmpa))
                tt(AL.mult, ent(u1), aE(0, 1), aE(2, 2))
                tt(AL.mult, ent(tmpa), aE(2, 1), aE(0, 2))
                tt(AL.subtract, ent(u1), ent(u1), ent(tmpa))
                tt(AL.mult, ent(u2), aE(0, 1), aE(1, 2))
                tt(AL.mult, ent(tmpa), aE(1, 1), aE(0, 2))
                tt(AL.subtract, ent(u2), ent(u2), ent(tmpa))
                tt(AL.mult, ent(detA), aE(0, 0), ent(u0))
                tt(AL.mult, ent(tmpa), aE(1, 0), ent(u1))
                tt(AL.subtract, ent(detA), ent(detA), ent(tmpa))
                tt(AL.mult, ent(tmpa), aE(2, 0), ent(u2))
                tt(AL.add, ent(detA), ent(detA), ent(tmpa))
                s.detA = detA

            def S5sin():
                s.sinL = wt(f"sinL_{qb}")
                act.activation(s.sinL[:, :], s.at[:, :], AF.Sin,
                               bias=bias_sinl[:, :], scale=-1.0 / 3.0)
                s.sinM = wt(f"sinM_{qb}")
                act.activation(s.sinM[:, :], s.at[:, :], AF.Sin,
                               scale=-1.0 / 3.0)

            def S6():
                """eigenvalues, gaps, clamps; issue the g/recip ACT block."""
                tmpb = wt(f"tmpe_{qb}")
                s.lam3, s.lam1 = xt(f"lam3_{qb}"), xt(f"lam1_{qb}")
                lam2 = wt(f"lam2_{qb}")
                tt(AL.mult, ent(tmpb), ent(s.two_pb), ent(s.sinL))
                tt(AL.add, ent(s.lam3), ent(s.qm), ent(tmpb))
                tt(AL.mult, ent(tmpb), ent(s.two_pb), ent(s.sinM))
                tt(AL.add, ent(lam2), ent(s.qm), ent(tmpb))
                tt(AL.subtract, ent(tmpb), ent(s.trb), ent(s.lam3))
                tt(AL.subtract, ent(s.lam1), ent(tmpb), ent(lam2))
                tt(AL.subtract, ent(tmpb), ent(s.sinL), ent(s.sinM))
                # d-quad = [d21, d31, ssum(later), d32]
                s.dq = wt(f"dq_{qb}", BF, 4, tag="q4", bufs=8)
                tt(AL.subtract, ent(s.dq, 0), ent(lam2), ent(s.lam1))
                tt(AL.subtract, ent(s.dq, 1), ent(s.lam3), ent(s.lam1))
                tt(AL.mult, ent(s.dq, 3), ent(s.two_pb), ent(tmpb))
                # l-pair = [max(lam2, clip), max(lam3, clip)]
                s.lp = wt(f"lp_{qb}", BF, 2)
                vec.tensor_scalar_max(out=s.lp[:, 0:FD], in0=lam2[:, :],
                                      scalar1=CLIPV)
                vec.tensor_scalar_max(out=s.lp[:, FD:2 * FD],
                                      in0=s.lam3[:, :], scalar1=CLIPV)
                # eps-quad = [l3q, l3q, l3c, l3q] (x^2-scales of dq slots)
                s.epsq = wt(f"epsq_{qb}", BF, 4, tag="q4", bufs=8)
                vec.tensor_copy(s.epsq[:, 2 * FD:3 * FD],
                                s.lp[:, FD:2 * FD])
                # ACT block (all natural_log_exp set): g-pair, l3^4
                lnl = wt(f"lnl_{qb}", BF, 2)
                act.activation(lnl[:, :], s.lp[:, :], AF.Ln)
                s.gP = wt(f"gP_{qb}", BF, 2)
                act.activation(s.gP[:, :], lnl[:, :], AF.Exp, scale=-0.5)
                l3sq = wt(f"l3sq_{qb}")
                act.square(l3sq[:, :], s.lp[:, FD:2 * FD])
                s.l3q = wt(f"l3q_{qb}")
                act.square(s.l3q[:, :], l3sq[:, :])

            def S7a():
                """[needs gP, l3q] ssum/q23/eps-quad; issue square(dq)."""
                s.sq3 = wt(f"sq3_{qb}")
                tt(AL.mult, ent(s.sq3), ent(s.lp, 1), ent(s.gP, 1))
                tmps = wt(f"tmps_{qb}")
                tt(AL.mult, ent(tmps), ent(s.lp, 0), ent(s.gP, 0))
                tt(AL.add, ent(s.dq, 2), ent(tmps), ent(s.sq3))
                s.q23 = wt(f"q23_{qb}")
                tt(AL.mult, ent(s.q23), ent(s.gP, 0), ent(s.gP, 1))
                s.dsg = xt(f"dsg_{qb}")
                act.sign(s.dsg[:, :], s.detA[:, :])
                s.selb = xt(f"selb_{qb}")
                act.activation(s.selb[:, :], s.dsg[:, :], AF.Copy, bias=0.5,
                               scale=0.5)
                lq = s.l3q[:, :]
                lqb = bass.AP(lq.tensor, lq.offset,
                              [list(lq.ap[0]), [0, 2], [1, FD]])
                dst01 = s.epsq[:, 0:2 * FD]
                vec.tensor_copy(bass.AP(dst01.tensor, dst01.offset,
                                        [list(dst01.ap[0]), [FD, 2],
                                         [1, FD]]), lqb)
                vec.tensor_copy(s.epsq[:, 3 * FD:4 * FD], lq)
                s.sqq = wt(f"sqq_{qb}", BF, 4, tag="q4", bufs=8)
                act.square(s.sqq[:, :], s.dq[:, :])

            def S7a2():
                """[needs sqq] eps-add + floor; issue ln/exp quad."""
                vec.scalar_tensor_tensor(out=s.sqq[:, :],
                                         in0=s.epsq[:, :],
                                         scalar=1e-12, in1=s.sqq[:, :],
                                         op0=AL.mult, op1=AL.add)
                vec.tensor_scalar_max(out=s.sqq[:, :], in0=s.sqq[:, :],
                                      scalar1=1e-30)
                act.activation(s.sqq[:, :], s.sqq[:, :], AF.Ln)
                s.eiq = wt(f"eiq_{qb}", BF, 4, tag="q4", bufs=8)
                act.activation(s.eiq[:, :], s.sqq[:, :], AF.Exp,
                               scale=-1.0)

            def S7b():
                """gam's, T2, W2, Y, R, energy residuals (big DVE block)."""
                # N1/Md/T2 first: независимы of the reciprocal quad, so the
                # DVE has work while ACT finishes the ln/exp for iq.
                N1p = wt(f"N1p_{qb}", BF, 6)
                tt(AL.subtract, tri(N1p, 0), tri(s.Cp, 0),
                   bc(ent(s.lam1), 3))
                vec.tensor_copy(N1p[:, 3 * FD:6 * FD],
                                s.Cp[:, 3 * FD:6 * FD])
                Md3 = wt(f"Md3_{qb}", BF, 3)
                tt(AL.subtract, tri(Md3), tri(s.Cp, 0), bc(ent(s.lam3), 3))
                T2p = wt(f"T2p_{qb}", BF, 6)
                tt(AL.mult, tri(T2p, 0), tri(N1p, 0), tri(Md3))
                tt(AL.add, ent(T2p, 0), ent(T2p, 0), s.sq01)
                tt(AL.add, ent(T2p, 0), ent(T2p, 0), s.sq02)
                tt(AL.add, ent(T2p, 1), ent(T2p, 1), s.sq01)
                tt(AL.add, ent(T2p, 1), ent(T2p, 1), s.sq12)
                tt(AL.add, ent(T2p, 2), ent(T2p, 2), s.sq02)
                tt(AL.add, ent(T2p, 2), ent(T2p, 2), s.sq12)
                tq = wt(f"tq_{qb}")
                for (slot, a, mslot, cslot, cpx) in (
                        (3, 0, 1, 3, s.cp01), (4, 0, 2, 4, s.cp02),
                        (5, 1, 2, 5, s.cp12)):
                    tt(AL.add, ent(tq), ent(N1p, a), ent(Md3, mslot))
                    tt(AL.mult, ent(T2p, slot), s.cE(cslot), ent(tq))
                    tt(AL.add, ent(T2p, slot), ent(T2p, slot), ent(cpx))
                # i-quad = [1/d21, 1/d31, 1/ssum, 1/d32]
                iq = s.eiq
                vec.tensor_tensor(out=iq[:, :], in0=s.dq[:, :],
                                  in1=iq[:, :], op=AL.mult)
                s.p2131 = wt(f"p2131_{qb}")
                tt(AL.mult, ent(s.p2131), ent(iq, 0), ent(iq, 1))
                c3, c4 = wt(f"c3_{qb}"), wt(f"c4_{qb}")
                t1 = wt(f"t1_{qb}")
                # c3+ = -(d21/ssum + sq3) * q23 * i21*i31
                tt(AL.mult, ent(t1), ent(s.dq, 0), ent(iq, 2))
                tt(AL.add, ent(t1), ent(t1), ent(s.sq3))
                tt(AL.mult, ent(t1), ent(t1), ent(s.q23))
                c3p = wt(f"c3p_{qb}")
                vec.scalar_tensor_tensor(out=ent(c3p), in0=ent(t1),
                                         scalar=-1.0, in1=ent(s.p2131),
                                         op0=AL.mult, op1=AL.mult)
                # c3- = -(g3*d21 + g2*d31) / (d21*d31*d32)
                t2 = wt(f"t2_{qb}")
                tt(AL.mult, ent(t2), ent(s.gP, 1), ent(s.dq, 0))
                tt(AL.mult, ent(t1), ent(s.gP, 0), ent(s.dq, 1))
                tt(AL.add, ent(t2), ent(t2), ent(t1))
                tt(AL.mult, ent(t2), ent(t2), ent(iq, 3))
                c3m = wt(f"c3m_{qb}")
                vec.scalar_tensor_tensor(out=ent(c3m), in0=ent(t2),
                                         scalar=-1.0, in1=ent(s.p2131),
                                         op0=AL.mult, op1=AL.mult)
                # blend on sign: c3 = c3m + (dsg+1)/2 * (c3p - c3m)
                selb = s.selb
                tt(AL.subtract, ent(t1), ent(c3p), ent(c3m))
                tt(AL.mult, ent(t1), ent(selb), ent(t1))
                tt(AL.add, ent(c3), ent(c3m), ent(t1))
                # c4 = dsg * g3 * i31
                tt(AL.mult, ent(c4), ent(s.gP, 1), ent(iq, 1))
                tt(AL.mult, ent(c4), ent(s.dsg), ent(c4))
                # W2 = c3*T2 + c4*N1 (in place on T2p)
                t6 = wt(f"t6_{qb}", BF, 6)
                tt(AL.mult, six(t6), six(N1p), bc(ent(c4), 6))
                tt(AL.mult, six(T2p), six(T2p), bc(ent(c3), 6))
                tt(AL.add, six(T2p), six(T2p), six(t6))
                w2 = lambda cc, j: bc(ent(T2p, SYM[(cc, j)]), 3)
                s.dbg_W2, s.dbg_c3, s.dbg_c4 = T2p, c3, c4
                # Y = A @ W2
                Yp = wt(f"Yp_{qb}", BF, 9)
                t3 = wt(f"t3b_{qb}", BF, 3)
                acol = lambda cc: tri(s.Ap, cc, stride=3 * FD)
                ycol = lambda j: tri(Yp, j, stride=3 * FD)
                for j in range(3):
                    tt(AL.mult, ycol(j), acol(0), w2(0, j))
                    for cc in (1, 2):
                        tt(AL.mult, tri(t3), acol(cc), w2(cc, j))
                        tt(AL.add, ycol(j), ycol(j), tri(t3))
                # R = Y + cof(Y)
                s.dbg_Yp = Yp
                Rp = wt(f"Rp_{qb}", BF, 9)
                s.dbg_Rp = Rp
                yE = lambda i, j: ent(Yp, i * 3 + j)
                cf = wt(f"cf_{qb}")
                tmpb = wt(f"tmpf_{qb}")
                for i in range(3):
                    for j in range(3):
                        i1, i2 = (i + 1) % 3, (i + 2) % 3
                        j1, j2 = (j + 1) % 3, (j + 2) % 3
                        tt(AL.mult, ent(cf), yE(i1, j1), yE(i2, j2))
                        tt(AL.mult, ent(tmpb), yE(i1, j2), yE(i2, j1))
                        tt(AL.subtract, ent(cf), ent(cf), ent(tmpb))
                        tt(AL.add, ent(Rp, i * 3 + j), yE(i, j), ent(cf))
                # energy residuals (software-pipelined with ACT squares)
                rcol = lambda j: tri(Rp, j, stride=3 * FD)
                Rpm3 = wt(f"Rpm3_{qb}", BF, 3)
                tt(AL.add, tri(Rpm3), rcol(0), rcol(1))
                Z3 = wt(f"Z3_{qb}", BF, 3)
                dfc = [wt(f"dfc{i}_{qb}", BF, 3) for i in (0, 1)]
                sqd = [wt(f"sqd{i}_{qb}", BF, 3) for i in (0, 1)]
                s.ns6 = wt(f"ns6_{qb}", BF, 6)
                combos = ((AL.add, tri(Rpm3)), (AL.add, rcol(0)),
                          (AL.add, rcol(1)), (AL.subtract, rcol(1)),
                          (AL.subtract, rcol(0)), (AL.subtract, tri(Rpm3)))

                def emit_dfc(k):
                    d = dfc[k % 2]
                    tt(AL.mult, tri(Z3), rcol(2), r_tz(k))
                    tt(AL.subtract, tri(d), s.eT(k), tri(Z3))
                    op, cv = combos[k]
                    tt(op, tri(d), tri(d), cv)
                    act.square(sqd[k % 2][:, :], d[:, :])

                def emit_ns(k):
                    sq = sqd[k % 2]
                    tt(AL.add, ent(s.ns6, k), ent(sq, 0), ent(sq, 1))
                    tt(AL.add, ent(s.ns6, k), ent(s.ns6, k), ent(sq, 2))

                emit_dfc(0)
                for k in range(1, K):
                    emit_dfc(k)
                    emit_ns(k - 1)
                emit_ns(K - 1)

            def S8sqrt():
                if dbg_n and qb == 0:
                    FDl = FD

                    def dump(slot, t, n):
                        nc.sync.dma_start(
                            dbg_d[:, slot * FDl:(slot + n) * FDl],
                            t[:, 0:n * FDl])
                    dump(0, s.Ap, 9)
                    dump(9, s.Cp, 6)
                    dump(15, s.gP, 2)
                    dump(17, s.dbg_c3, 1)
                    dump(18, s.dbg_c4, 1)
                    dump(19, s.lam1, 1)
                    dump(20, s.lam3, 1)
                    dump(21, s.dbg_W2, 6)
                    dump(27, s.dbg_Yp, 9)
                    dump(36, s.dbg_Rp, 9)
                    dump(45, s.Et, 3)
                    dump(48, s.dsg, 1)
                    dump(49, s.ns6, 6)
                    dump(55, s.dp, 2)
                    dump(57, s.dsq, 2)
                    dump(59, s.ei, 2)
                    dump(61, s.l3q, 1)
                act.activation(s.ns6[:, :], s.ns6[:, :], AF.Sqrt)

            def S8():
                tt(AL.mult, six(s.ns6), six(s.ns6), wrow6())
                s3t = wt(f"s3_{qb}", BF, 3)
                tt(AL.add, tri(s3t), tri(s.ns6, 0), tri(s.ns6, 3))
                nrg = wt(f"nrg_{qb}")
                tt(AL.add, ent(nrg), ent(s3t, 0), ent(s3t, 1))
                tt(AL.add, ent(nrg), ent(nrg), ent(s3t, 2))
                vec.tensor_scalar_min(out=nrg[:, :], in0=nrg[:, :],
                                      scalar1=1.0)
                vec.tensor_reduce(out=outacc[:, qb * BQ:(qb + 1) * BQ],
                                  in_=ent(nrg), axis=mybir.AxisListType.X,
                                  op=AL.add)

            return [S0a, S1, S2, S3, S4, S5s, S5at, S5sin, S6, S7a,
                    S7a2, S7b, S8sqrt, S8]

        for q0 in range(0, NQ, 2):
            segsA = build_pass(q0)
            segsB = build_pass(q0 + 1)
            for sa, sb in zip(segsA, segsB):
                sa()
                sb()

        nc.sync.dma_start(out_d[:, :], outacc[:, :])

    nc.compile()
    return nc


def _get_nc(wingeo):
    if wingeo not in _nc_cache:
        _pin_act_tables()
        _nc_cache[wingeo] = _build_nc(wingeo)
    return _nc_cache[wingeo]


# ---------------------------------------------------------------------------
# Entry point
# ---------------------------------------------------------------------------

def _install_ntff_shim():
    """Provide antenv.axon_hooks (missing in this image) so
    run_bass_kernel_spmd(trace=True) can reach the NTFF profiler in
    libaxon_pjrt.so."""
    import types

    try:
        import antenv.axon_hooks  # noqa: F401
        return True
    except ImportError:
        pass
    try:
        import antenv
        from trn_agent_boot.trn_boot import _ntff_profile_via_ctypes
    except ImportError:
        return False
    mod = types.ModuleType("antenv.axon_hooks")
    state = {"hook": None}
    mod.set_axon_ntff_profile_hook = lambda h: state.__setitem__("hook", h)
    mod.get_axon_ntff_profile_hook = lambda: state["hook"]
    sys.modules["antenv.axon_hooks"] = mod
    antenv.axon_hooks = mod
    try:
        hook = _ntff_profile_via_ctypes("/opt/axon/libaxon_pjrt.so")
    except OSError:
        hook = None
    if hook is not None:
        mod.set_axon_ntff_profile_hook(hook)
    return hook is not None


def kernel(**inputs) -> np.ndarray:
    pred = np.asarray(inputs["prediction"], np.float32)
    adj_idx = np.asarray(inputs["adj_list_indices"])
    adj_w = np.asarray(inputs["adj_list_weights"], np.float32)
    tev_T = np.asarray(inputs["template_edge_vectors_T"], np.float32)

    wk, tzk = _build_offset_classes(adj_idx, adj_w, tev_T)
    in_maps, wingeo = _host_prepare(pred, wk, tzk)

    nc = _get_nc(wingeo)
    import os
    trace = bool(int(os.environ.get("ARAP_TRACE", "0")))
    if trace:
        trace = _install_ntff_shim()
    try:
        res = run_bass_kernel_spmd(nc, in_maps, core_ids=list(range(NCORES)),
                                   trace=trace)
    except Exception:
        if not trace:
            raise
        res = run_bass_kernel_spmd(nc, in_maps, core_ids=list(range(NCORES)),
                                   trace=False)
    kernel._last_exec_ns = res.exec_time_ns
    kernel._last_results = res

    total = np.zeros(B, np.float64)
    for c in range(NCORES):
        total += res.results[c]["out"].astype(np.float64).sum(axis=0)
    return (total / NV).astype(np.float32)


kernel._last_exec_ns = None
